# revision 98
# baseline (speedup 1.0000x reference)
"""Trainium2 Bass kernel for nn_DecoderLayer (B=2, S=2048, D=512, H=8, FH=2048).

Sharding: 8 cores = 2 batches x 4 query-blocks of 512 tokens.  Each core
computes its 512 output tokens end-to-end (K/V projections for the full
batch are recomputed on each core; no collectives).  Host rolls x/y per
core so the core's query block occupies rows 0..511 -- attention is
permutation-invariant in the key/value token order, so K/V built from the
rolled sequence give exact results as long as any nonzero attention mask
columns are rolled identically.

Precision: attention projections + the softmax-weighted sum run in
fp8e4m3 with DoubleRow matmuls (half cycles-per-row, 2x contraction per
pass; weights are host-cast, halving/quartering their HBM traffic); the
attention context is scaled by 64 on the way into fp8 to clear the
denormal range and unscaled in the out-projection drain.  The FFN runs in
bf16 (its error lands directly on the output through the residual), and
residuals + layernorms stay fp32.  Scores are fp32r.

On-device layout: activations are feature-major [D on partitions (chunks
of 128), tokens on free dim]; V is fp8 token-major in key-chunk-pair
tiles [P, 2, H, VWP] with a ones column per head (the softmax denominator
rides along as row 64 of the DoubleRow attention matmul); attention
scores are computed directly transposed (t_k on partitions) in [P, 2T]
double chunks whose halves are adjacent key chunks = the DoubleRow planes
of the fp8 exp output.  Softmax reciprocals run on the vector engine; all
scalar-engine functions are steered into one activation-table set (single
ACT_TABLE_LOAD).

The phases are scalar-engine(exp)-bound, so cross-attention prep (x
transposes, K projection) and the tail of the self V projection are
interleaved into the self-attention phase as PE filler work, and the 6MB
FFN weight load is prefetched under cross-attention.
"""

import sys
from contextlib import ExitStack

for _p in ("/opt/trn_rl_repo",):
    if _p not in sys.path:
        sys.path.insert(0, _p)

import numpy as np

import concourse.bass as bass
import concourse.tile as tile
from concourse import bacc, mybir
from concourse import bass_utils
from concourse.bass import ts
from concourse.masks import make_identity

F32 = mybir.dt.float32
F32R = mybir.dt.float32r
F8 = mybir.dt.float8e4
BF16 = mybir.dt.bfloat16
DR = mybir.MatmulPerfMode.DoubleRow
AF = mybir.ActivationFunctionType
ALU = mybir.AluOpType

B, S, D, H, FH = 2, 2048, 512, 8, 2048
HD = D // H          # 64
T = 512              # query tokens per core
P = 128
DC = D // P          # 4 feature chunks
SC = S // P          # 16 sequence token chunks
TC = T // P          # 4 query token chunks
FC = FH // P         # 16 ffn-hidden chunks
N_CORES = 8
EPS = 1e-5
ISCALE = 1.0 / 8.0   # 1/sqrt(HD)
VW = HD + 1          # per-head V block width (v columns + ones column)
VWP = 72             # padded per-head V width (16B-aligned DoubleRow plane step)

# Steer every scalar-engine activation to the one table set that contains all
# functions this kernel uses (exp, ln, identity, copy, relu, square), so only
# a single ACT_TABLE_LOAD is ever emitted.  Set ids stay aligned with
# act_info.json (walrus resolves ids by original index); we only stop the
# other sets from matching.
import concourse.hw_specs as _hw_specs

_KEEP_SET = "natural_log_exp_and_others"
_STEER_FUNCS = {AF.Exp, AF.Ln, AF.Identity, AF.Copy, AF.Relu, AF.Square}


def _steered_activation_tables(arch):
    tables = _hw_specs.get_activation_tables(arch)
    return {
        name: (funcs if name == _KEEP_SET else (funcs - _STEER_FUNCS))
        for name, funcs in tables.items()
    }


bacc.get_activation_tables = _steered_activation_tables


def _mm(nc, out, lhsT, rhs, **kw):
    """fp32r matmul: out (+)= lhsT.T @ rhs (inputs are fp32 APs)."""
    nc.tensor.matmul(out, lhsT.bitcast(F32R), rhs.bitcast(F32R), **kw)


def _r(ap):
    """Round-on-write view: walrus requires instructions whose output feeds
    an fp32r matmul to write fp32r (fp22-rounded) values."""
    return ap.bitcast(F32R)


def _load_w8(nc, pool, dram_ap, d_in, cols, name):
    """Load fp8 [d_in, cols] weight as one [P, d_in//P, cols] tile (d_in
    chunk pairs become DoubleRow planes)."""
    nch = d_in // P
    t = pool.tile([P, nch * cols], F8, name=name)
    nc.sync.dma_start(
        t[:].rearrange("p (c m) -> p c m", c=nch),
        dram_ap.rearrange("(c p) m -> p c m", p=P),
    )
    return t


def _v3(t, nch):
    """[P, nch*cols] tile -> [P, nch, cols] view."""
    return t[:].rearrange("p (c m) -> p c m", c=nch)


def _transpose_in(nc, tc_, tm_pool, ps_pool, out8, dram_ap, n_rows, name,
                  ps_tag="tp", res_tiles=None):
    """DRAM token-major [n_rows, D] -> feature-major fp8 SBUF tile out8
    ([P, DC, n_rows] view) via PE transposes.  res_tiles: optional DC x
    [P, T] f32 tiles receiving the first T token columns (residual path)."""
    ident = tc_.ident
    o3 = _v3(out8, DC)
    for sc in range(n_rows // P):
        tm = tm_pool.tile([P, D], F32, name=f"{name}_tm{sc}", tag=f"{name}_tm")
        nc.sync.dma_start(tm[:], dram_ap[ts(sc, P), :])
        for dc in range(DC):
            tp = ps_pool.tile([P, P], F32, name=f"{name}_tp{sc}_{dc}", tag=ps_tag)
            nc.tensor.transpose(tp[:], tm[:, ts(dc, P)], ident[:])
            nc.vector.tensor_copy(o3[:, dc, ts(sc, P)], tp[:])
            if res_tiles is not None and sc < TC:
                nc.vector.tensor_copy(res_tiles[dc][:, ts(sc, P)], tp[:])


def _transpose_mask(nc, tc_, sb_pool, dram_ap, name):
    """DRAM [T, S] mask -> SC tiles of [P(t_k), T(t_q)] (transposed)."""
    ident = tc_.ident
    out = [
        sb_pool.tile([P, T], F32, name=f"{name}_mT{j}", tag=f"{name}_mT", bufs=SC)
        for j in range(SC)
    ]
    with tc_.tile_pool(name=f"{name}_mtm", bufs=2) as mtm, \
         tc_.tile_pool(name=f"{name}_mps", bufs=4, space="PSUM") as mps:
        for tc2 in range(TC):
            tmm = mtm.tile([P, S], F32, name=f"{name}_tm{tc2}", tag="mtm")
            nc.sync.dma_start(tmm[:], dram_ap[ts(tc2, P), :])
            for j in range(SC):
                tp = mps.tile([P, P], F32, name=f"{name}_tp{tc2}_{j}", tag="tp")
                nc.tensor.transpose(tp[:], tmm[:, ts(j, P)], ident[:])
                nc.vector.tensor_copy(out[j][:, ts(tc2, P)], tp[:])
    return out


def _feat_layernorm(nc, tc_, sb, ps, r_tiles, out_tiles, g_ap, b_ap, tag):
    """Feature-major layernorm over D (partitions, DC chunks), one result per
    free-dim token column.  Per-token stats are computed replicated across all
    128 partitions via all-ones lhsT matmuls."""
    ones = tc_.ones
    s1 = ps.tile([P, T], F32, name=f"{tag}_s1", tag="pj")
    s2 = ps.tile([P, T], F32, name=f"{tag}_s2", tag="pj")
    sq_tiles = []
    for c in range(DC):
        sq = sb.tile([P, T], F32, name=f"{tag}_sq{c}", tag="lnsq", bufs=2 * DC)
        nc.vector.tensor_mul(_r(sq[:]), r_tiles[c][:], r_tiles[c][:])
        sq_tiles.append(sq)
    for c in range(DC):
        _mm(nc, s1[:], ones[:], r_tiles[c][:], start=(c == 0), stop=(c == DC - 1))
    for c in range(DC):
        _mm(nc, s2[:], ones[:], sq_tiles[c][:], start=(c == 0), stop=(c == DC - 1))
    s1_sb = sb.tile([P, T], F32, name=f"{tag}_s1sb", tag="lntmp", bufs=2)
    nc.vector.tensor_copy(s1_sb[:], s1[:])
    m2 = sb.tile([P, T], F32, name=f"{tag}_m2", tag="lntmp2", bufs=3)
    nc.vector.tensor_mul(m2[:], s1_sb[:], s1_sb[:])
    # u = s2 - s1^2/D   (then var = u/(D-1): Bessel-corrected)
    u = sb.tile([P, T], F32, name=f"{tag}_u", tag="lntmp2", bufs=3)
    nc.vector.scalar_tensor_tensor(u[:], m2[:], -1.0 / D, s2[:], ALU.mult, ALU.add)
    # rstd = 1/sqrt(var) = exp(-0.5*ln(u/(D-1))).  (The reference adds
    # eps=1e-5 to a std of ~1 before inverting -- a 1e-5 relative shift,
    # negligible vs the fp8 budget -- so the sqrt/add/reciprocal spine
    # collapses into one activation.)
    lnv = sb.tile([P, T], F32, name=f"{tag}_lnv", tag="lntmp2", bufs=3)
    nc.scalar.activation(lnv[:], u[:], AF.Ln, scale=1.0 / (D - 1))
    rstd = sb.tile([P, T], F32, name=f"{tag}_rstd", tag="lntmp", bufs=2)
    nc.scalar.activation(rstd[:], lnv[:], AF.Exp, scale=-0.5)
    for c in range(DC):
        cen = sb.tile([P, T], F32, name=f"{tag}_cen{c}", tag="lnsq", bufs=2 * DC)
        nc.vector.scalar_tensor_tensor(
            cen[:], s1_sb[:], -1.0 / D, r_tiles[c][:], ALU.mult, ALU.add
        )
        if g_ap is not None:
            nc.vector.tensor_mul(out_tiles[c][:], cen[:], rstd[:])
            nc.vector.tensor_scalar(
                _r(out_tiles[c][:]), out_tiles[c][:],
                g_ap[c], b_ap[c], ALU.mult, ALU.add,
            )
        else:
            nc.vector.tensor_mul(_r(out_tiles[c][:]), cen[:], rstd[:])


def _attention(nc, tc_, sb, ptp, ps, kT, V, qT, maskT, out8, tag,
               filler=()):
    """Multi-head attention.  kT: DC x [P, S] feature-major keys; V:
    (SC//2) x [P, 2*H*VWP] fp8 token-major values, logically [P, 2(key
    chunk), H, VWP] with per-head layout [v_h | 1 | pad]; qT: DC x [P, T]
    feature-major queries.  maskT: None or SC x [P, T] (mask transposed).
    out8: [P, DC*T] fp8 feature-major normalized attention output scaled
    by 64 (head 2p -> rows 0:64, head 2p+1 -> rows 64:128 of column block
    p; the odd head is lane-migrated via a small SBUF-SBUF DMA since
    matmul outputs must sit at partition base 0).

    Head pairs share the key-chunk loop so the two K=64 score matmuls land
    in different PE row groups.  Scores/exp are processed in [P, 2T]
    double chunks (the two halves are ADJACENT KEY CHUNKS for all T
    queries); exp writes fp8 so the softmax-weighted sum runs as a single
    fp8 DoubleRow matmul per double-chunk (contraction 256 keys at half
    cycles-per-row: 4x fewer PE cycles than per-chunk fp32r).  Each head's
    softmax epilogue is pipelined one pair behind so the PE FIFO never
    waits on ACT.  `filler` is an iterable of closures emitting
    independent PE work (cross-attn prep) to fill ACT-bound gaps."""
    ones = tc_.ones
    pending = []
    fill_iter = iter(filler)

    def epilogue(h, av):
        # 1/sum on lane 64 (DVE reciprocal keeps the scalar engine free for
        # exp); copy the raw attention rows out of PSUM now so the av bank
        # frees for the next pair.  The normalize multiplies by an extra 64
        # to keep the fp8 context out of the denormal range; the out-proj
        # drain divides it back out.
        rec = sb.tile([P, T], F32, name=f"{tag}_rec{h}", tag="smrec", bufs=3)
        with nc.allow_low_precision(reason="fp32r view of an fp32 reciprocal"):
            nc.vector.reciprocal(_r(rec[64:65, :]), av[64:65, :])
        raw = sb.tile([64, T], F32, name=f"{tag}_raw{h}", tag="raw", bufs=3)
        nc.vector.tensor_copy(raw[:], av[0:64, :])

        def finish():
            pair, sub = h // 2, h % 2
            bc = ps.tile([P, T], F32, name=f"{tag}_bc{h}", tag="pj", bufs=2)
            _mm(nc, bc[0:64, :], ones[64:65, 0:64], rec[64:65, :])
            if sub == 0:
                nc.vector.scalar_tensor_tensor(
                    out8[0:64, ts(pair, T)], raw[:], 64.0, bc[0:64, :],
                    ALU.mult, ALU.mult,
                )
            else:
                # normalize at lanes 0..63, then DMA-migrate to lanes 64..127
                tmp = sb.tile([64, T], F8, name=f"{tag}_mig{h}", tag="omig",
                              bufs=1)
                nc.vector.scalar_tensor_tensor(
                    tmp[:], raw[:], 64.0, bc[0:64, :], ALU.mult, ALU.mult)
                nc.sync.dma_start(out8[64:128, ts(pair, T)], tmp[:])

        return finish

    for pair in range(H // 2):
        h0 = 2 * pair
        avs = [
            ps.tile([P, T], F32, name=f"{tag}_av{h0 + sub}", tag="av", bufs=2)
            for sub in range(2)
        ]
        for dj in range(SC // 2):
            sts = [
                ps.tile([P, 2 * T], F32, name=f"{tag}_st{h0 + sub}_{dj}",
                        tag="st", bufs=2)
                for sub in range(2)
            ]
            for half in range(2):
                j = 2 * dj + half
                for sub in range(2):
                    rb = 64 * sub
                    _mm(
                        nc, sts[sub][:, ts(half, T)],
                        kT[pair][rb:rb + 64, ts(j, P)],
                        qT[pair][rb:rb + 64, :],
                    )
                    if maskT is not None:
                        nc.vector.scalar_tensor_tensor(
                            sts[sub][:, ts(half, T)], sts[sub][:, ts(half, T)],
                            ISCALE, maskT[j][:], ALU.mult, ALU.add,
                        )
            for sub in range(2):
                pt = ptp.tile([P, 2 * T], F8, name=f"{tag}_pt{h0 + sub}_{dj}",
                              tag="pt", bufs=4)
                if maskT is not None:
                    nc.scalar.activation(pt[:], sts[sub][:], AF.Exp)
                else:
                    nc.scalar.activation(pt[:], sts[sub][:], AF.Exp,
                                         scale=ISCALE)
                h = h0 + sub
                vsl = V[dj].rearrange("p (k h x) -> p k h x", k=2, h=H)
                nc.tensor.matmul(
                    avs[sub][0:VW, :],
                    vsl[:, :, h, 0:VW],
                    pt[:].rearrange("p (k t) -> p k t", k=2),
                    start=(dj == 0), stop=(dj == SC // 2 - 1),
                    perf_mode=DR,
                )
            if dj == 1:
                for fin in pending:
                    fin()
                pending = []
            nxt = next(fill_iter, None)
            if nxt is not None:
                nxt()
        pending = [epilogue(h0, avs[0]), epilogue(h0 + 1, avs[1])]
    for fin in pending:
        fin()
    for nxt in fill_iter:
        nxt()


def build_program(flags, repeat=1):
    """Build and compile the Bass program.  flags keys: bias_qkv,
    bias_self_out, bias_kv, bias_q, bias_cross_out, bias_f1, bias_f2,
    ln1, ln2, ln3, mask_self, mask_cross.  repeat>1 unrolls the body
    multiple times (benchmarking only: amortizes dispatch overhead)."""
    nc = bacc.Bacc(
        "TRN2", target_bir_lowering=False, debug=False,
        num_devices=1, enable_asserts=False,
    )
    x_d = nc.dram_tensor("x", [S, D], F32, kind="ExternalInput").ap()
    y_d = nc.dram_tensor("y", [S, D], F32, kind="ExternalInput").ap()
    # weights arrive host-cast to fp8e4m3
    w_qkv = nc.dram_tensor("qkv_w", [D, 3 * D], F8, kind="ExternalInput").ap()
    w_so = nc.dram_tensor("self_out_w", [D, D], F8, kind="ExternalInput").ap()
    w_kv = nc.dram_tensor("kv_w", [D, 2 * D], F8, kind="ExternalInput").ap()
    w_q = nc.dram_tensor("q_w", [D, D], F8, kind="ExternalInput").ap()
    w_co = nc.dram_tensor("cross_out_w", [D, D], F8, kind="ExternalInput").ap()
    w_f1 = nc.dram_tensor("ffn_w1", [D, FH], BF16, kind="ExternalInput").ap()
    w_f2 = nc.dram_tensor("ffn_w2", [FH, D], BF16, kind="ExternalInput").ap()

    def opt_in(name, shape, flag):
        if flags[flag]:
            return nc.dram_tensor(name, shape, F32, kind="ExternalInput").ap()
        return None

    b_qkv_d = opt_in("qkv_b", [3 * D], "bias_qkv")
    b_so_d = opt_in("self_out_b", [D], "bias_self_out")
    b_kv_d = opt_in("kv_b", [2 * D], "bias_kv")
    b_q_d = opt_in("q_b", [D], "bias_q")
    b_co_d = opt_in("cross_out_b", [D], "bias_cross_out")
    b_f1_d = opt_in("ffn_b1", [FH], "bias_f1")
    b_f2_d = opt_in("ffn_b2", [D], "bias_f2")
    g1_d = opt_in("g1", [D], "ln1")
    b1_d = opt_in("b1", [D], "ln1")
    g2_d = opt_in("g2", [D], "ln2")
    b2_d = opt_in("b2", [D], "ln2")
    g3_d = opt_in("g3", [D], "ln3")
    b3_d = opt_in("b3", [D], "ln3")
    m_self_d = opt_in("mask_self", [T, S], "mask_self")
    m_cross_d = opt_in("mask_cross", [T, S], "mask_cross")

    out_d = nc.dram_tensor("out", [T, D], F32, kind="ExternalOutput").ap()

    with tile.TileContext(nc, pool_alloc_mode="queue") as tc_:
      for _rep in range(repeat):
       with ExitStack() as top:
        persist = top.enter_context(tc_.tile_pool(name="persist", bufs=1))

        ident = persist.tile([P, P], F32, name="ident")
        make_identity(nc, ident[:])
        ones_raw = persist.tile([P, P], F32, name="ones_raw")
        nc.vector.memset(ones_raw[:], 1.0)
        ones = persist.tile([P, P], F32, name="ones")
        nc.vector.tensor_copy(_r(ones[:]), ones_raw[:])
        tc_.ident = ident
        tc_.ones = ones
        tc_.ones_raw = ones_raw
        # dummy activation: forces the one ACT_TABLE_LOAD to run at t=0,
        # under the input DMA, instead of on the first-exp critical path
        actwarm = persist.tile([1, 1], F32, name="actwarm")
        nc.scalar.activation(actwarm[:], ones_raw[0:1, 0:1], AF.Exp)

        def load_vec_chunks(dram_ap, n, name):
            """[n] DRAM vector -> SBUF [P, n//P] (col c = chunk c)."""
            if dram_ap is None:
                return None
            t = persist.tile([P, n // P], F32, name=name)
            nc.sync.dma_start(t[:], dram_ap.rearrange("(c p) -> p c", p=P))
            return t

        b_qkv = load_vec_chunks(b_qkv_d, 3 * D, "b_qkv")
        b_so = load_vec_chunks(b_so_d, D, "b_so")
        b_kv = load_vec_chunks(b_kv_d, 2 * D, "b_kv")
        b_q = load_vec_chunks(b_q_d, D, "b_q")
        b_co = load_vec_chunks(b_co_d, D, "b_co")
        b_f1 = load_vec_chunks(b_f1_d, FH, "b_f1")
        b_f2 = load_vec_chunks(b_f2_d, D, "b_f2")
        g1 = load_vec_chunks(g1_d, D, "g1")
        b1 = load_vec_chunks(b1_d, D, "b1")
        g2 = load_vec_chunks(g2_d, D, "g2")
        b2 = load_vec_chunks(b2_d, D, "b2")
        g3 = load_vec_chunks(g3_d, D, "g3")
        b3 = load_vec_chunks(b3_d, D, "b3")

        y1_tiles = [persist.tile([P, T], F32, name=f"y1_{c}") for c in range(DC)]

        def gb_cols(g, b):
            if g is None:
                return None, None
            return (
                [g[:, c:c + 1] for c in range(DC)],
                [b[:, c:c + 1] for c in range(DC)],
            )

        def bias_bcast(sb_pool, ps_pool, src_ap, name):
            """Bias row (any AP of D elements in head order) broadcast across
            partitions -> [P, D]."""
            row = sb_pool.tile([P, D], F32, name=f"{name}_row")
            nc.sync.dma_start(_r(row[0:1, :]), _r(src_ap))
            bc_ps = ps_pool.tile([P, D], F32, name=f"{name}_ps", tag="pj")
            _mm(nc, bc_ps[:], ones[0:1, :], row[0:1, :])
            out = sb_pool.tile([P, D], F32, name=f"{name}_bc")
            nc.vector.tensor_copy(out[:], bc_ps[:])
            return out

        def proj_kT(ps, src8, w8, w_ncols, w_off, kT_o, bk_cols, tag, mc, ns,
                    drain):
            """One [P, T] tile of the feature-major K projection (fp8 DR)."""
            kp = ps.tile([P, T], F32, name=f"{tag}_kp{mc}_{ns}", tag="pj")
            s3 = _v3(src8, DC)
            w3 = _v3(w8, DC)
            for jp in range(DC // 2):
                nc.tensor.matmul(
                    kp[:],
                    w3[:, 2 * jp:2 * jp + 2,
                       w_off + mc * P:w_off + mc * P + P],
                    s3[:, 2 * jp:2 * jp + 2, ts(ns, T)],
                    start=(jp == 0), stop=(jp == DC // 2 - 1), perf_mode=DR,
                )
            dst = _r(kT_o[mc][:, ts(ns, T)])
            if drain == "act":
                if bk_cols is not None:
                    nc.scalar.activation(dst, kp[:], AF.Identity,
                                         bias=bk_cols[mc])
                else:
                    nc.scalar.copy(dst, kp[:])
            else:
                if bk_cols is not None:
                    nc.vector.tensor_scalar_add(dst, kp[:], bk_cols[mc])
                else:
                    nc.vector.tensor_copy(dst, kp[:])

        def v_ones_init(V_o):
            """Write the per-head softmax-denominator ones column of every V
            tile once (a cheap strided memset; the value drains never touch
            column HD)."""
            for v8 in V_o:
                nc.vector.memset(
                    v8[:].rearrange("p (k h x) -> p k h x", k=2, h=H)
                    [:, :, :, HD:HD + 1], 1.0)

        def proj_V(ps, src8, w8, v_off, V_o, vb_bc, tag, tc2, drain="dve"):
            """One key chunk of the fp8 V tiles: plane tc2%2 of pair tile
            tc2//2 (logical [P, 2, H, VWP]; ones column pre-written by
            v_ones_init)."""
            vp = ps.tile([P, D], F32, name=f"{tag}_vp{tc2}", tag="pj")
            s3 = _v3(src8, DC)
            w3 = _v3(w8, DC)
            for jp in range(DC // 2):
                nc.tensor.matmul(
                    vp[:],
                    s3[:, 2 * jp:2 * jp + 2, ts(tc2, P)],
                    w3[:, 2 * jp:2 * jp + 2, v_off:v_off + D],
                    start=(jp == 0), stop=(jp == DC // 2 - 1), perf_mode=DR,
                )
            vdst = V_o[tc2 // 2].rearrange(
                "p (k h x) -> p k h x", k=2, h=H)[:, tc2 % 2]
            if vb_bc is not None:
                nc.vector.tensor_add(
                    vdst[:, :, 0:HD], vp[:].rearrange("p (h x) -> p h x", h=H),
                    vb_bc[:].rearrange("p (h x) -> p h x", h=H),
                )
            elif drain == "act":
                nc.scalar.copy(
                    vdst[:, :, 0:HD], vp[:].rearrange("p (h x) -> p h x", h=H))
            else:
                nc.vector.tensor_copy(
                    vdst[:, :, 0:HD], vp[:].rearrange("p (h x) -> p h x", h=H))

        def proj_qT(ps, src8, src_ncols, w8, q_off, qT_o, bq_cols, tag,
                    mcs=None):
            s3 = _v3(src8, src_ncols)
            w3 = _v3(w8, DC)
            for mc in (range(DC) if mcs is None else mcs):
                qp = ps.tile([P, T], F32, name=f"{tag}_qp{mc}", tag="pj")
                for jp in range(DC // 2):
                    nc.tensor.matmul(
                        qp[:],
                        w3[:, 2 * jp:2 * jp + 2,
                           q_off + mc * P:q_off + mc * P + P],
                        s3[:, 2 * jp:2 * jp + 2, 0:T],
                        start=(jp == 0), stop=(jp == DC // 2 - 1),
                        perf_mode=DR,
                    )
                if bq_cols is not None:
                    nc.scalar.activation(_r(qT_o[mc][:]), qp[:], AF.Identity,
                                         bias=bq_cols[mc])
                else:
                    nc.scalar.copy(_r(qT_o[mc][:]), qp[:])

        def out_proj_residual(ps_blk, w8, attn8, bias, resid, r_out):
            a3 = _v3(attn8, DC)
            w3 = _v3(w8, DC)
            for mc in range(DC):
                op = ps_blk.tile([P, T], F32, name=f"op{mc}", tag="pj")
                for jp in range(DC // 2):
                    nc.tensor.matmul(
                        op[:],
                        w3[:, 2 * jp:2 * jp + 2, ts(mc, P)],
                        a3[:, 2 * jp:2 * jp + 2, :],
                        start=(jp == 0), stop=(jp == DC // 2 - 1),
                        perf_mode=DR,
                    )
                # attention context was written scaled by 64 (fp8 range);
                # undo it here while adding the residual.
                nc.vector.scalar_tensor_tensor(
                    _r(r_out[mc][:]), op[:], 1.0 / 64.0,
                    resid[mc][:], ALU.mult, ALU.add,
                )
                if bias is not None:
                    nc.vector.tensor_scalar_add(
                        _r(r_out[mc][:]), r_out[mc][:], bias[:, mc:mc + 1])

        # ==================== SELF-ATTENTION BLOCK ====================
        sa_kv_blk = ExitStack()
        sa_sb = top.enter_context(tc_.tile_pool(name="sa_sb", bufs=1))
        sa_kv = sa_kv_blk.enter_context(tc_.tile_pool(name="sa_kv", bufs=1))
        kT_s = [sa_kv.tile([P, S], F32, name=f"kTs{c}") for c in range(DC)]
        V_s = [sa_kv.tile([P, 2 * H * VWP], F8, name=f"Vs{j}")
               for j in range(SC // 2)]
        qT_s = [sa_kv.tile([P, T], F32, name=f"qTs{c}") for c in range(DC)]
        attn_p_s = sa_sb.tile([P, DC * T], F8, name="attnPs")
        y_res = [sa_sb.tile([P, T], F32, name=f"yres{c}") for c in range(DC)]
        maskT_s = None
        if m_self_d is not None:
            maskT_s = _transpose_mask(nc, tc_, sa_kv, m_self_d, "ms")

        yT8 = sa_kv.tile([P, DC * S], F8, name="yT8")
        # qkv_w/qkv_b arrive host-permuted to [all-q | all-k | all-v],
        # head-major inside each section -> contiguous slices here.  The K
        # section loads first: it heads the first-score critical chain.
        wq8 = sa_kv.tile([P, DC * 3 * D], F8, name="wqkv8")
        _wq3d = wq8[:].rearrange("p (c m) -> p c m", c=DC)
        _wqd3 = w_qkv.rearrange("(c p) m -> p c m", p=P)
        nc.sync.dma_start(_wq3d[:, :, D:2 * D], _wqd3[:, :, D:2 * D])
        vb_bc = None
        bk_cols = bq_cols = None
        if b_qkv is not None:
            with tc_.tile_pool(name="vb_ps", bufs=1, space="PSUM") as vps:
                vb_bc = bias_bcast(
                    sa_sb, vps, b_qkv_d[2 * D:3 * D].rearrange("(a n) -> a n", a=1),
                    "vb_s")
            bk_cols = [b_qkv[:, DC + mc:DC + mc + 1] for mc in range(DC)]
            bq_cols = [b_qkv[:, mc:mc + 1] for mc in range(DC)]
        no_mask = m_self_d is None and m_cross_d is None
        v_ones_init(V_s)
        if no_mask:
            # Minimal pre-attention prefix: only what pair 0's first two
            # double-chunks need (y tokens 0:512 transposed, kT(pair0, first
            # 512 keys), all queries, V pairs 0-1).  Everything else streams
            # in as deadline-ordered filler slots inside the attention phase,
            # so the first exp fires after ~1.75MB of DMA instead of ~5MB.
            with tc_.tile_pool(name="sa_tm", bufs=4) as tm_pool, \
                 tc_.tile_pool(name="sa_tp", bufs=4, space="PSUM") as tp_ps:
                o3 = _v3(yT8, DC)
                for sc in range(TC):
                    tm = tm_pool.tile([P, D], F32, name=f"y_tm{sc}",
                                      tag="y_tm")
                    nc.sync.dma_start(tm[:], y_d[ts(sc, P), :])
                    for dc in range(DC):
                        tp = tp_ps.tile([P, P], F32, name=f"y_tp{sc}_{dc}",
                                        tag="tp")
                        nc.tensor.transpose(tp[:], tm[:, ts(dc, P)], ident[:])
                        nc.vector.tensor_copy(o3[:, dc, ts(sc, P)], tp[:])
                        if sc < TC:
                            nc.vector.tensor_copy(
                                y_res[dc][:, ts(sc, P)], tp[:])
            nc.sync.dma_start(_wq3d[:, :, 0:D], _wqd3[:, :, 0:D])
            nc.sync.dma_start(_wq3d[:, :, 2 * D:3 * D], _wqd3[:, :, 2 * D:3 * D])
            with tc_.tile_pool(name="sa_prj_ps", bufs=4, space="PSUM") as ps:
                proj_kT(ps, yT8, wq8, 3 * D, D, kT_s, bk_cols, "sa", 0, 0,
                        drain="act")
                proj_qT(ps, yT8, DC, wq8, 0, qT_s, bq_cols, "sa")
                for tc2 in range(4):
                    proj_V(ps, yT8, wq8, 2 * D, V_s, vb_bc, "sa", tc2)
        else:
            with tc_.tile_pool(name="sa_tm", bufs=4) as tm_pool, \
                 tc_.tile_pool(name="sa_tp", bufs=4, space="PSUM") as tp_ps:
                _transpose_in(nc, tc_, tm_pool, tp_ps, yT8, y_d, S, "y",
                              res_tiles=y_res)
            with tc_.tile_pool(name="sa_prj_ps", bufs=4, space="PSUM") as ps:
                for mc in range(DC):
                    for ns in range(SC // DC):
                        proj_kT(ps, yT8, wq8, 3 * D, D, kT_s, bk_cols, "sa",
                                mc, ns, drain="act")
                proj_qT(ps, yT8, DC, wq8, 0, qT_s, bq_cols, "sa")
                for tc2 in range(SC):
                    proj_V(ps, yT8, wq8, 2 * D, V_s, vb_bc, "sa", tc2)

        # Cross-attention prep (x transposes + cross K projection) is
        # independent of self-attention; in the no-mask variant it is
        # interleaved into the self-attention phase as PE filler work.
        xw_blk = ExitStack()
        ca_kT_blk = ExitStack()
        _cross_prep = {}

        def setup_cross_prep():
            ca_kT_pool = ca_kT_blk.enter_context(
                tc_.tile_pool(name="ca_kT", bufs=1, side="right"))
            kT_c = [ca_kT_pool.tile([P, S], F32, name=f"kTc{c}")
                    for c in range(DC)]
            xw_sb = xw_blk.enter_context(
                tc_.tile_pool(name="xw_sb", bufs=1, side="right"))
            xT8 = xw_sb.tile([P, DC * S], F8, name="xT8")
            wkv8 = _load_w8(nc, xw_sb, w_kv, D, 2 * D, "wkv8")
            _cross_prep["kT_c"] = kT_c
            _cross_prep["xT8"] = xT8
            _cross_prep["wkv8"] = wkv8
            return kT_c, xT8, wkv8

        bk_cols_c = None
        if b_kv is not None:
            bk_cols_c = [b_kv[:, mc:mc + 1] for mc in range(DC)]

        def make_filler(ptp, aps):
            """32 filler slots (one consumed at the end of each attention
            double-chunk iteration).  Slot s is emitted before iteration s+1,
            so every unit sits ahead of its first consumer in the PE queue:
            y-transpose sc before kT(*, sc//4) / V(sc//2) in the same or a
            later slot; V pair j before pair-0 AV of chunk j (iteration j);
            kT(p, ns) before pair p's scores on keys ns*512.. (iteration
            8p+2ns); x-transposes/cross-K only feed the (later) cross
            attention."""
            kT_c, xT8 = _cross_prep["kT_c"], _cross_prep["xT8"]
            wkv8 = _cross_prep["wkv8"]
            xtm_pool = ptp  # token-major staging tiles share the pt pool
            x3 = _v3(xT8, DC)
            y3 = _v3(yT8, DC)

            def ytr(sc):
                def emit():
                    tm = xtm_pool.tile([P, D], F32, name=f"y_tm{sc}",
                                       tag="xtm", bufs=4)
                    nc.sync.dma_start(tm[:], y_d[ts(sc, P), :])
                    for dc in range(DC):
                        tp = aps.tile([P, P], F32, name=f"y_tp{sc}_{dc}",
                                      tag="pj", bufs=2)
                        nc.tensor.transpose(tp[:], tm[:, ts(dc, P)],
                                            tc_.ident[:])
                        nc.vector.tensor_copy(y3[:, dc, ts(sc, P)], tp[:])
                return emit

            def vs(jp):
                def emit():
                    proj_V(aps, yT8, wq8, 2 * D, V_s, vb_bc, "sa", 2 * jp)
                    proj_V(aps, yT8, wq8, 2 * D, V_s, vb_bc, "sa", 2 * jp + 1)
                return emit

            def kts(mc, ns):
                def emit():
                    proj_kT(aps, yT8, wq8, 3 * D, D, kT_s, bk_cols, "sa",
                            mc, ns, drain="dve")
                return emit

            def qts(mc):
                def emit():
                    proj_qT(aps, yT8, DC, wq8, 0, qT_s, bq_cols, "sa",
                            mcs=[mc])
                return emit

            def xtr(sc):
                def emit():
                    tm = xtm_pool.tile([P, D], F32, name=f"x_tm{sc}",
                                       tag="xtm", bufs=4)
                    nc.sync.dma_start(tm[:], x_d[ts(sc, P), :])
                    for dc in range(DC):
                        tp = aps.tile([P, P], F32, name=f"x_tp{sc}_{dc}",
                                      tag="pj", bufs=2)
                        nc.tensor.transpose(tp[:], tm[:, ts(dc, P)],
                                            tc_.ident[:])
                        nc.vector.tensor_copy(x3[:, dc, ts(sc, P)], tp[:])
                return emit

            def kc(mc, ns):
                def emit():
                    proj_kT(aps, xT8, wkv8, 2 * D, 0, kT_c, bk_cols_c, "ca",
                            mc, ns, drain="dve")
                return emit

            # within a slot: score-critical kT first (its DVE drain must
            # not queue behind the V drains), transposes next, V last; the
            # y-transpose stream is front-loaded so each kT lands 1-2 slots
            # before its consumer instead of exactly at the deadline
            slots = [
                [ytr(4), ytr(5), ytr(6), ytr(7), vs(2)],
                [kts(0, 1), ytr(8), ytr(9), ytr(10), ytr(11), vs(3)],
                [kts(0, 2), ytr(12), ytr(13), ytr(14), ytr(15), vs(4)],
                [kts(0, 3), vs(5)],
                [kts(1, 0), vs(6)],
                [kts(1, 1), vs(7)],
                [kts(1, 2)],
                [kts(1, 3)],
                [xtr(0), xtr(1)],
                [xtr(2), xtr(3)],
                [kts(2, 0), xtr(4), xtr(5)],
                [kts(2, 1), xtr(6), xtr(7)],
                [kts(2, 2), xtr(8), xtr(9)],
                [kts(2, 3), xtr(10), xtr(11)],
                [kts(3, 0), xtr(12), xtr(13)],
                [kts(3, 1), xtr(14), xtr(15)],
                [kts(3, 2), kc(0, 0)],
                [kts(3, 3), kc(1, 0)],
                [kc(2, 0)], [kc(3, 0)],
                [kc(0, 1)], [kc(1, 1)], [kc(2, 1)], [kc(3, 1)],
                [kc(0, 2)], [kc(1, 2)], [kc(2, 2)], [kc(3, 2)],
                [kc(0, 3)], [kc(1, 3)], [kc(2, 3)], [kc(3, 3)],
            ]

            def slotfn(lst):
                def emit():
                    for u in lst:
                        u()
                return emit

            return [slotfn(lst) for lst in slots]

        # prefetch the small out-proj / cross-q weights so the inter-phase
        # stretch never waits on their DMA
        wso8 = _load_w8(nc, sa_sb, w_so, D, D, "wso8")
        wqc8 = _load_w8(nc, sa_sb, w_q, D, D, "wqc8")
        wco8 = _load_w8(nc, sa_sb, w_co, D, D, "wco8")

        if m_cross_d is None and m_self_d is None:
            kT_c, xT8, wkv8 = setup_cross_prep()
            with tc_.tile_pool(name="att_s_sb", bufs=1) as asb, \
                 tc_.tile_pool(name="att_s_pt", bufs=1) as aptp, \
                 tc_.tile_pool(name="att_s_ps", bufs=2, space="PSUM") as aps:
                _attention(nc, tc_, asb, aptp, aps, kT_s, V_s, qT_s, maskT_s,
                           attn_p_s, "sa", filler=make_filler(aptp, aps))
        else:
            # mask variant: no interleaving (SBUF budget goes to mask tiles)
            with tc_.tile_pool(name="att_s_sb", bufs=1) as asb, \
                 tc_.tile_pool(name="att_s_pt", bufs=1) as aptp, \
                 tc_.tile_pool(name="att_s_ps", bufs=2, space="PSUM") as aps:
                _attention(nc, tc_, asb, aptp, aps, kT_s, V_s, qT_s, maskT_s,
                           attn_p_s, "sa")
            sa_kv_blk.close()
            kT_c, xT8, wkv8 = setup_cross_prep()
            with tc_.tile_pool(name="xp_tm", bufs=4) as tm_pool, \
                 tc_.tile_pool(name="xp_ps", bufs=4, space="PSUM") as tp_ps:
                _transpose_in(nc, tc_, tm_pool, tp_ps, xT8, x_d, S, "x")
                for mc in range(DC):
                    for ns in range(SC // DC):
                        proj_kT(tp_ps, xT8, wkv8, 2 * D, 0, kT_c, bk_cols_c,
                                "ca", mc, ns, drain="dve")

        sa_kv_blk.close()
        y2_pool = top.enter_context(tc_.tile_pool(name="y2_pool", bufs=1))
        y2_tiles = [y2_pool.tile([P, T], F32, name=f"y2_{c}") for c in range(DC)]

        # Cross V projection + pools hoisted before LN1: its matmuls fill
        # the PE gap while LN1's scalar/vector chain runs (separate PSUM
        # pool avoids the bank-reuse WAR with LN1's stats tiles).
        blk = ExitStack()
        ca_sb = blk.enter_context(tc_.tile_pool(name="ca_sb", bufs=1))
        ca_v_sb = blk.enter_context(tc_.tile_pool(name="ca_v_sb", bufs=1))
        V_c = [ca_v_sb.tile([P, 2 * H * VWP], F8, name=f"Vc{j}")
               for j in range(SC // 2)]
        qT_c = [ca_sb.tile([P, T], F32, name=f"qTc{c}") for c in range(DC)]
        attn_p_c = ca_sb.tile([P, DC * T], F8, name="attnPc")
        maskT_c = None
        vb_bc_c = None
        bq_cols_c = None
        if b_kv is not None:
            with tc_.tile_pool(name="vbc_ps", bufs=1, space="PSUM") as vps:
                vb_bc_c = bias_bcast(
                    ca_sb, vps, b_kv_d[D:2 * D].rearrange("(a n) -> a n", a=1),
                    "vb_c")
        if b_q is not None:
            bq_cols_c = [b_q[:, mc:mc + 1] for mc in range(DC)]
        ca_ps_blk = ExitStack()
        ca_prj_ps = ca_ps_blk.enter_context(
            tc_.tile_pool(name="ca_prj_ps", bufs=4, space="PSUM"))
        v_ones_init(V_c)
        for tc2 in range(SC):
            proj_V(ca_prj_ps, xT8, wkv8, D, V_c, vb_bc_c, "ca", tc2,
                   drain="act")
        xw_blk.close()

        with tc_.tile_pool(name="ph4_sb", bufs=1) as p4sb, \
             tc_.tile_pool(name="ph4_ps", bufs=2, space="PSUM") as p4ps:
            r1 = [p4sb.tile([P, T], F32, name=f"r1_{c}") for c in range(DC)]
            out_proj_residual(p4ps, wso8, attn_p_s, b_so, y_res, r1)
            g1c, b1c = gb_cols(g1, b1)
            _feat_layernorm(nc, tc_, p4sb, p4ps, r1, y1_tiles, g1c, b1c, "ln1")
            y18 = ca_sb.tile([P, DC * T], F8, name="y18")
            for c in range(DC):
                eng = nc.scalar.copy if c % 2 == 0 else nc.vector.tensor_copy
                eng(_v3(y18, DC)[:, c, :], y1_tiles[c][:])
            proj_qT(ca_prj_ps, y18, DC, wqc8, 0, qT_c, bq_cols_c, "ca")
        ca_ps_blk.close()

        # FFN weights prefetch: the 6MB bf16 load overlaps the (scalar-
        # engine-bound) cross-attention phase instead of stalling the tail.
        # Allocated from the long-lived y2_pool to keep pool stack order.
        wf1b = y2_pool.tile([P, DC * FH], BF16, name="wf1b")
        nc.sync.dma_start(
            _v3(wf1b, DC), w_f1.rearrange("(c p) m -> p c m", p=P))
        wf2b = y2_pool.tile([P, FC * D], BF16, name="wf2b")
        nc.sync.dma_start(
            _v3(wf2b, FC), w_f2.rearrange("(c p) m -> p c m", p=P))

        # ==================== CROSS-ATTENTION BLOCK ====================
        if True:
            if m_cross_d is not None:
                mc_pool = blk.enter_context(
                    tc_.tile_pool(name="mc_pool", bufs=1))
                maskT_c = _transpose_mask(nc, tc_, mc_pool, m_cross_d, "mc")

            with tc_.tile_pool(name="att_c_sb", bufs=1) as asb, \
                 tc_.tile_pool(name="att_c_pt", bufs=1) as aptp, \
                 tc_.tile_pool(name="att_c_ps", bufs=2, space="PSUM") as aps:
                _attention(nc, tc_, asb, aptp, aps, kT_c, V_c, qT_c, maskT_c,
                           attn_p_c, "ca")
            ca_kT_blk.close()

            with tc_.tile_pool(name="ph5d_sb", bufs=1) as p5sb, \
                 tc_.tile_pool(name="ph5d_ps", bufs=2, space="PSUM") as p5ps:
                r2 = [p5sb.tile([P, T], F32, name=f"r2_{c}") for c in range(DC)]
                out_proj_residual(p5ps, wco8, attn_p_c, b_co, y1_tiles, r2)
                g2c, b2c = gb_cols(g2, b2)
                _feat_layernorm(nc, tc_, p5sb, p5ps, r2, y2_tiles, g2c, b2c, "ln2")
        blk.close()

        # ==================== FFN + LN3 + STORE ====================
        # FFN runs in fp32r: same PE rate as bf16 at these tile widths, but
        # the LN2 output feeds the matmuls directly (no cast on the critical
        # chain) and the rounding error is ~fp22 instead of bf16.
        with tc_.tile_pool(name="ph6_sb", bufs=1) as p6sb, \
             tc_.tile_pool(name="ph6_ps", bufs=2, space="PSUM") as p6ps:
            y2b = p6sb.tile([P, DC * T], BF16, name="y2b")
            for c in range(DC):
                eng = nc.scalar.copy if c % 2 == 0 else nc.vector.tensor_copy
                eng(_v3(y2b, DC)[:, c, :], y2_tiles[c][:])
            y2b3 = _v3(y2b, DC)
            wf13 = _v3(wf1b, DC)
            wf23 = _v3(wf2b, FC)
            hb = p6sb.tile([P, FC * T], BF16, name="hb")
            hb3 = _v3(hb, FC)
            for fc in range(FC):
                hp = p6ps.tile([P, T], F32, name=f"hps{fc}", tag="pj")
                for kc in range(DC):
                    nc.tensor.matmul(
                        hp[:],
                        wf13[:, kc, ts(fc, P)],
                        y2b3[:, kc, :],
                        start=(kc == 0), stop=(kc == DC - 1),
                    )
                if b_f1 is not None:
                    nc.scalar.activation(
                        hb3[:, fc, :], hp[:], AF.Relu, bias=b_f1[:, fc:fc + 1]
                    )
                else:
                    nc.scalar.activation(hb3[:, fc, :], hp[:], AF.Relu)
            r3 = [p6sb.tile([P, T], F32, name=f"r3_{c}") for c in range(DC)]
            for mc in range(DC):
                op = p6ps.tile([P, T], F32, name=f"fop{mc}", tag="pj")
                for fc in range(FC):
                    nc.tensor.matmul(
                        op[:],
                        wf23[:, fc, ts(mc, P)],
                        hb3[:, fc, :],
                        start=(fc == 0), stop=(fc == FC - 1),
                    )
                if b_f2 is not None:
                    nc.vector.scalar_tensor_tensor(
                        r3[mc][:], op[:], b_f2[:, mc:mc + 1], y2_tiles[mc][:],
                        ALU.add, ALU.add,
                    )
                else:
                    nc.vector.tensor_add(r3[mc][:], op[:], y2_tiles[mc][:])

            # LN3 token-major: transpose r3, bn_stats, normalize, store
            # (pipelines per token chunk; beats a feature-major pass here).
            g3bc = b3bc = None
            if g3 is not None:
                g3row = p6sb.tile([P, D], F32, name="g3row")
                nc.sync.dma_start(_r(g3row[0:1, :]), _r(g3_d.rearrange("(a n) -> a n", a=1)))
                b3row = p6sb.tile([P, D], F32, name="b3row")
                nc.sync.dma_start(_r(b3row[0:1, :]), _r(b3_d.rearrange("(a n) -> a n", a=1)))
                g3ps = p6ps.tile([P, D], F32, name="g3ps", tag="pj")
                _mm(nc, g3ps[:], ones[0:1, :], g3row[0:1, :])
                g3bc = p6sb.tile([P, D], F32, name="g3bc")
                nc.vector.tensor_copy(g3bc[:], g3ps[:])
                b3ps = p6ps.tile([P, D], F32, name="b3ps", tag="pj")
                _mm(nc, b3ps[:], ones[0:1, :], b3row[0:1, :])
                b3bc = p6sb.tile([P, D], F32, name="b3bc")
                nc.vector.tensor_copy(b3bc[:], b3ps[:])

            for tc2 in range(TC):
                rtm = p6sb.tile([P, D], F32, name=f"rtm{tc2}", tag="rtm", bufs=3)
                for dc in range(DC):
                    tp = p6ps.tile([P, P], F32, name=f"otp{tc2}_{dc}", tag="otp")
                    nc.tensor.transpose(tp[:], r3[dc][:, ts(tc2, P)], ident[:])
                    nc.vector.tensor_copy(rtm[:, ts(dc, P)], tp[:])
                st6 = p6sb.tile([P, 6], F32, name=f"st6_{tc2}", tag="st6", bufs=3)
                nc.vector.bn_stats(st6[:], rtm[:])
                mv = p6sb.tile([P, 2], F32, name=f"mv{tc2}", tag="mv", bufs=3)
                nc.vector.bn_aggr(mv[:], st6[:])
                # rstd = 1/sqrt(var*D/(D-1)) (eps fold: see _feat_layernorm)
                lnv = p6sb.tile([P, 1], F32, name=f"olnv{tc2}", tag="osm", bufs=3)
                nc.scalar.activation(
                    lnv[:], mv[:, 1:2], AF.Ln, scale=float(D) / (D - 1)
                )
                rstd = p6sb.tile([P, 1], F32, name=f"orstd{tc2}", tag="osm4", bufs=3)
                nc.scalar.activation(rstd[:], lnv[:], AF.Exp, scale=-0.5)
                otm = p6sb.tile([P, D], F32, name=f"otm{tc2}", tag="otm", bufs=3)
                nc.vector.tensor_scalar(
                    otm[:], rtm[:], mv[:, 0:1], rstd[:], ALU.subtract, ALU.mult
                )
                if g3bc is not None:
                    nc.vector.tensor_mul(otm[:], otm[:], g3bc[:])
                    nc.vector.tensor_add(otm[:], otm[:], b3bc[:])
                nc.sync.dma_start(out_d[ts(tc2, P), :], otm[:])

    nc.compile()
    return nc


_PROGRAM_CACHE = {}


def _get_program(flags):
    key = tuple(sorted(flags.items()))
    if key not in _PROGRAM_CACHE:
        _PROGRAM_CACHE[key] = build_program(flags)
    return _PROGRAM_CACHE[key]


def make_flags(inputs):
    def nz(name):
        return bool(np.any(np.asarray(inputs[name])))

    return {
        "bias_qkv": nz("qkv_b"),
        "bias_self_out": nz("self_out_b"),
        "bias_kv": nz("kv_b"),
        "bias_q": nz("q_b"),
        "bias_cross_out": nz("cross_out_b"),
        "bias_f1": nz("ffn_b1"),
        "bias_f2": nz("ffn_b2"),
        "ln1": bool(np.any(np.asarray(inputs["g1"]) != 1.0) or nz("b1")),
        "ln2": bool(np.any(np.asarray(inputs["g2"]) != 1.0) or nz("b2")),
        "ln3": bool(np.any(np.asarray(inputs["g3"]) != 1.0) or nz("b3")),
        "mask_self": nz("self_attention_mask"),
        "mask_cross": nz("cross_attention_mask"),
    }


def make_in_maps(inputs, flags):
    """Per-core input dicts.  Core c handles batch c//4, query block c%4;
    x/y are rolled so the query block is first."""
    inputs = {k: np.asarray(v) for k, v in inputs.items()}
    # The reference splits qkv per head (reshape(B,S,H,3*HD) then split on the
    # last axis), i.e. qkv_w columns are [q_h|k_h|v_h] blocks of HD per head.
    # Permute on the host to [all-q | all-k | all-v] (head-major inside each
    # section) so the device uses contiguous slices.
    qkv_w = inputs["qkv_w"].reshape(D, H, 3, HD).transpose(0, 2, 1, 3).reshape(D, 3 * D)
    qkv_b = inputs["qkv_b"].reshape(H, 3, HD).transpose(1, 0, 2).reshape(3 * D)
    kv_w = inputs["kv_w"].reshape(D, H, 2, HD).transpose(0, 2, 1, 3).reshape(D, 2 * D)
    kv_b = inputs["kv_b"].reshape(H, 2, HD).transpose(1, 0, 2).reshape(2 * D)
    inputs = {**inputs, "qkv_w": qkv_w, "qkv_b": qkv_b, "kv_w": kv_w, "kv_b": kv_b}
    # attention matmuls run in fp8e4m3 DoubleRow, the FFN in bf16: cast the
    # weights on the host (this also cuts weight HBM traffic 4x / 2x).
    f8 = mybir.dt.np(F8)
    bf16 = mybir.dt.np(BF16)
    for wname in ("qkv_w", "kv_w", "q_w", "self_out_w", "cross_out_w"):
        inputs[wname] = np.asarray(inputs[wname], np.float32).astype(f8)
    for wname in ("ffn_w1", "ffn_w2"):
        inputs[wname] = np.asarray(inputs[wname], np.float32).astype(bf16)
    in_maps = []
    for c in range(N_CORES):
        b, qb = c // 4, c % 4
        qoff = qb * T
        m = {
            "x": np.roll(inputs["x"][b], -qoff, axis=0),
            "y": np.roll(inputs["y"][b], -qoff, axis=0),
            "qkv_w": inputs["qkv_w"],
            "self_out_w": inputs["self_out_w"],
            "kv_w": inputs["kv_w"],
            "q_w": inputs["q_w"],
            "cross_out_w": inputs["cross_out_w"],
            "ffn_w1": inputs["ffn_w1"],
            "ffn_w2": inputs["ffn_w2"],
        }
        for flag, names in (
            ("bias_qkv", ["qkv_b"]), ("bias_self_out", ["self_out_b"]),
            ("bias_kv", ["kv_b"]), ("bias_q", ["q_b"]),
            ("bias_cross_out", ["cross_out_b"]),
            ("bias_f1", ["ffn_b1"]), ("bias_f2", ["ffn_b2"]),
            ("ln1", ["g1", "b1"]), ("ln2", ["g2", "b2"]), ("ln3", ["g3", "b3"]),
        ):
            if flags[flag]:
                for n in names:
                    m[n] = inputs[n]
        if flags["mask_self"]:
            msk = np.broadcast_to(inputs["self_attention_mask"], (1, 1, S, S))[0, 0]
            m["mask_self"] = np.roll(msk[qoff:qoff + T, :], -qoff, axis=1)
        if flags["mask_cross"]:
            msk = np.broadcast_to(inputs["cross_attention_mask"], (1, 1, S, S))[0, 0]
            m["mask_cross"] = np.roll(msk[qoff:qoff + T, :], -qoff, axis=1)
        m = {
            k: (np.ascontiguousarray(v) if v.dtype in (f8, bf16)
                else np.ascontiguousarray(v, dtype=np.float32))
            for k, v in m.items()
        }
        in_maps.append(m)
    return in_maps


def assemble_output(results):
    out = np.empty((B, S, D), np.float32)
    for c in range(N_CORES):
        b, qb = c // 4, c % 4
        out[b, qb * T:(qb + 1) * T, :] = results[c]["out"]
    return out


def kernel(**inputs) -> np.ndarray:
    flags = make_flags(inputs)
    nc = _get_program(flags)
    in_maps = make_in_maps(inputs, flags)
    last_err = None
    for attempt in range(3):
        try:
            res = bass_utils.run_bass_kernel_spmd(
                nc, in_maps, core_ids=list(range(N_CORES))
            )
            return assemble_output(res.results)
        except Exception as e:  # transient NRT device errors observed on axon
            last_err = e
            if "UNRECOVERABLE" not in str(e) and "UNAVAILABLE" not in str(e):
                raise
    raise last_err



# revision 102
# speedup vs baseline: 1.5042x; 1.5042x over previous
"""Trainium2 Bass kernel for nn_DecoderLayer (B=2, S=2048, D=512, H=8, FH=2048).

Sharding: 8 cores = 2 batches x 4 query-blocks of 512 tokens.  Each core
computes its 512 output tokens end-to-end (K/V projections for the full
batch are recomputed on each core; no collectives).  Host rolls x/y per
core so the core's query block occupies rows 0..511 -- attention is
permutation-invariant in the key/value token order, so K/V built from the
rolled sequence give exact results as long as any nonzero attention mask
columns are rolled identically.

Precision: attention projections + the softmax-weighted sum run in
fp8e4m3 with DoubleRow matmuls (half cycles-per-row, 2x contraction per
pass; weights are host-cast, halving/quartering their HBM traffic); the
attention context is scaled by 64 on the way into fp8 to clear the
denormal range and unscaled in the out-projection drain.  The FFN runs in
bf16 (its error lands directly on the output through the residual), and
residuals + layernorms stay fp32.  Scores are fp32r.

On-device layout: activations are feature-major [D on partitions (chunks
of 128), tokens on free dim]; V is fp8 token-major in key-chunk-pair
tiles [P, 2, H, VWP] with a ones column per head (the softmax denominator
rides along as row 64 of the DoubleRow attention matmul); attention
scores are computed directly transposed (t_k on partitions) in [P, 2T]
double chunks whose halves are adjacent key chunks = the DoubleRow planes
of the fp8 exp output.  Softmax reciprocals run on the vector engine; all
scalar-engine functions are steered into one activation-table set (single
ACT_TABLE_LOAD).

The phases are scalar-engine(exp)-bound, so cross-attention prep (x
transposes, K projection) and the tail of the self V projection are
interleaved into the self-attention phase as PE filler work, and the 6MB
FFN weight load is prefetched under cross-attention.
"""

import sys
from contextlib import ExitStack

for _p in ("/opt/trn_rl_repo",):
    if _p not in sys.path:
        sys.path.insert(0, _p)

import numpy as np

import concourse.bass as bass
import concourse.tile as tile
from concourse import bacc, mybir
from concourse import bass_utils
from concourse.bass import ts
from concourse.masks import make_identity

F32 = mybir.dt.float32
F32R = mybir.dt.float32r
F8 = mybir.dt.float8e4
BF16 = mybir.dt.bfloat16
DR = mybir.MatmulPerfMode.DoubleRow
AF = mybir.ActivationFunctionType
ALU = mybir.AluOpType

B, S, D, H, FH = 2, 2048, 512, 8, 2048
HD = D // H          # 64
T = 512              # query tokens per core
P = 128
DC = D // P          # 4 feature chunks
SC = S // P          # 16 sequence token chunks
TC = T // P          # 4 query token chunks
FC = FH // P         # 16 ffn-hidden chunks
N_CORES = 8
EPS = 1e-5
ISCALE = 1.0 / 8.0   # 1/sqrt(HD)
VW = HD + 1          # per-head V block width (v columns + ones column)
VWP = 72             # padded per-head V width (16B-aligned DoubleRow plane step)

# Steer every scalar-engine activation to the one table set that contains all
# functions this kernel uses (exp, ln, identity, copy, relu, square), so only
# a single ACT_TABLE_LOAD is ever emitted.  Set ids stay aligned with
# act_info.json (walrus resolves ids by original index); we only stop the
# other sets from matching.
import concourse.hw_specs as _hw_specs

_KEEP_SET = "natural_log_exp_and_others"
_STEER_FUNCS = {AF.Exp, AF.Ln, AF.Identity, AF.Copy, AF.Relu, AF.Square}


def _steered_activation_tables(arch):
    tables = _hw_specs.get_activation_tables(arch)
    return {
        name: (funcs if name == _KEEP_SET else (funcs - _STEER_FUNCS))
        for name, funcs in tables.items()
    }


bacc.get_activation_tables = _steered_activation_tables


def _mm(nc, out, lhsT, rhs, **kw):
    """fp32r matmul: out (+)= lhsT.T @ rhs (inputs are fp32 APs)."""
    nc.tensor.matmul(out, lhsT.bitcast(F32R), rhs.bitcast(F32R), **kw)


def _r(ap):
    """Round-on-write view: walrus requires instructions whose output feeds
    an fp32r matmul to write fp32r (fp22-rounded) values."""
    return ap.bitcast(F32R)


def _load_w8(nc, pool, dram_ap, d_in, cols, name):
    """Load fp8 [d_in, cols] weight as one [P, d_in//P, cols] tile (d_in
    chunk pairs become DoubleRow planes)."""
    nch = d_in // P
    t = pool.tile([P, nch * cols], F8, name=name)
    nc.sync.dma_start(
        t[:].rearrange("p (c m) -> p c m", c=nch),
        dram_ap.rearrange("(c p) m -> p c m", p=P),
    )
    return t


def _v3(t, nch):
    """[P, nch*cols] tile -> [P, nch, cols] view."""
    return t[:].rearrange("p (c m) -> p c m", c=nch)


def _transpose_in(nc, tc_, tm_pool, ps_pool, out8, dram_ap, n_rows, name,
                  ps_tag="tp", res_tiles=None):
    """DRAM token-major [n_rows, D] -> feature-major fp8 SBUF tile out8
    ([P, DC, n_rows] view) via PE transposes.  res_tiles: optional DC x
    [P, T] f32 tiles receiving the first T token columns (residual path)."""
    ident = tc_.ident
    o3 = _v3(out8, DC)
    for sc in range(n_rows // P):
        tm = tm_pool.tile([P, D], F32, name=f"{name}_tm{sc}", tag=f"{name}_tm")
        nc.sync.dma_start(tm[:], dram_ap[ts(sc, P), :])
        for dc in range(DC):
            tp = ps_pool.tile([P, P], F32, name=f"{name}_tp{sc}_{dc}", tag=ps_tag)
            nc.tensor.transpose(tp[:], tm[:, ts(dc, P)], ident[:])
            nc.vector.tensor_copy(o3[:, dc, ts(sc, P)], tp[:])
            if res_tiles is not None and sc < TC:
                nc.vector.tensor_copy(res_tiles[dc][:, ts(sc, P)], tp[:])


def _transpose_mask(nc, tc_, sb_pool, dram_ap, name):
    """DRAM [T, S] mask -> SC tiles of [P(t_k), T(t_q)] (transposed)."""
    ident = tc_.ident
    out = [
        sb_pool.tile([P, T], F32, name=f"{name}_mT{j}", tag=f"{name}_mT", bufs=SC)
        for j in range(SC)
    ]
    with tc_.tile_pool(name=f"{name}_mtm", bufs=2) as mtm, \
         tc_.tile_pool(name=f"{name}_mps", bufs=4, space="PSUM") as mps:
        for tc2 in range(TC):
            tmm = mtm.tile([P, S], F32, name=f"{name}_tm{tc2}", tag="mtm")
            nc.sync.dma_start(tmm[:], dram_ap[ts(tc2, P), :])
            for j in range(SC):
                tp = mps.tile([P, P], F32, name=f"{name}_tp{tc2}_{j}", tag="tp")
                nc.tensor.transpose(tp[:], tmm[:, ts(j, P)], ident[:])
                nc.vector.tensor_copy(out[j][:, ts(tc2, P)], tp[:])
    return out


def _feat_layernorm(nc, tc_, sb, ps, r_tiles, out_tiles, g_ap, b_ap, tag):
    """Feature-major layernorm over D (partitions, DC chunks), one result per
    free-dim token column.  Per-token stats are computed replicated across all
    128 partitions via all-ones lhsT matmuls."""
    ones = tc_.ones
    s1 = ps.tile([P, T], F32, name=f"{tag}_s1", tag="pj")
    s2 = ps.tile([P, T], F32, name=f"{tag}_s2", tag="pj")
    sq_tiles = []
    for c in range(DC):
        sq = sb.tile([P, T], F32, name=f"{tag}_sq{c}", tag="lnsq", bufs=2 * DC)
        nc.vector.tensor_mul(_r(sq[:]), r_tiles[c][:], r_tiles[c][:])
        sq_tiles.append(sq)
    for c in range(DC):
        _mm(nc, s1[:], ones[:], r_tiles[c][:], start=(c == 0), stop=(c == DC - 1))
    for c in range(DC):
        _mm(nc, s2[:], ones[:], sq_tiles[c][:], start=(c == 0), stop=(c == DC - 1))
    s1_sb = sb.tile([P, T], F32, name=f"{tag}_s1sb", tag="lntmp", bufs=2)
    nc.vector.tensor_copy(s1_sb[:], s1[:])
    m2 = sb.tile([P, T], F32, name=f"{tag}_m2", tag="lntmp2", bufs=3)
    nc.vector.tensor_mul(m2[:], s1_sb[:], s1_sb[:])
    # u = s2 - s1^2/D   (then var = u/(D-1): Bessel-corrected)
    u = sb.tile([P, T], F32, name=f"{tag}_u", tag="lntmp2", bufs=3)
    nc.vector.scalar_tensor_tensor(u[:], m2[:], -1.0 / D, s2[:], ALU.mult, ALU.add)
    # rstd = 1/sqrt(var) = exp(-0.5*ln(u/(D-1))).  (The reference adds
    # eps=1e-5 to a std of ~1 before inverting -- a 1e-5 relative shift,
    # negligible vs the fp8 budget -- so the sqrt/add/reciprocal spine
    # collapses into one activation.)
    lnv = sb.tile([P, T], F32, name=f"{tag}_lnv", tag="lntmp2", bufs=3)
    nc.scalar.activation(lnv[:], u[:], AF.Ln, scale=1.0 / (D - 1))
    rstd = sb.tile([P, T], F32, name=f"{tag}_rstd", tag="lntmp", bufs=2)
    nc.scalar.activation(rstd[:], lnv[:], AF.Exp, scale=-0.5)
    for c in range(DC):
        cen = sb.tile([P, T], F32, name=f"{tag}_cen{c}", tag="lnsq", bufs=2 * DC)
        nc.vector.scalar_tensor_tensor(
            cen[:], s1_sb[:], -1.0 / D, r_tiles[c][:], ALU.mult, ALU.add
        )
        if g_ap is not None:
            nc.vector.tensor_mul(out_tiles[c][:], cen[:], rstd[:])
            nc.vector.tensor_scalar(
                _r(out_tiles[c][:]), out_tiles[c][:],
                g_ap[c], b_ap[c], ALU.mult, ALU.add,
            )
        else:
            nc.vector.tensor_mul(_r(out_tiles[c][:]), cen[:], rstd[:])


def _attention(nc, tc_, sb, ptp, ps, kT, V, qT, maskT, out8, tag,
               filler=()):
    """Multi-head attention.  kT: DC x [P, S] feature-major keys; V:
    (SC//2) x [P, 2*H*VWP] fp8 token-major values, logically [P, 2(key
    chunk), H, VWP] with per-head layout [v_h | 1 | pad]; qT: DC x [P, T]
    feature-major queries.  maskT: None or SC x [P, T] (mask transposed).
    out8: [P, DC*T] fp8 feature-major normalized attention output scaled
    by 64 (head 2p -> rows 0:64, head 2p+1 -> rows 64:128 of column block
    p; the odd head is lane-migrated via a small SBUF-SBUF DMA since
    matmul outputs must sit at partition base 0).

    Head pairs share the key-chunk loop so the two K=64 score matmuls land
    in different PE row groups.  Scores/exp are processed in [P, 2T]
    double chunks (the two halves are ADJACENT KEY CHUNKS for all T
    queries); exp writes fp8 so the softmax-weighted sum runs as a single
    fp8 DoubleRow matmul per double-chunk (contraction 256 keys at half
    cycles-per-row: 4x fewer PE cycles than per-chunk fp32r).  Each head's
    softmax epilogue is pipelined one pair behind so the PE FIFO never
    waits on ACT.  `filler` is an iterable of closures emitting
    independent PE work (cross-attn prep) to fill ACT-bound gaps."""
    ones = tc_.ones
    pending = []
    fill_iter = iter(filler)

    def epilogue(h, av):
        # 1/sum on lane 64 (DVE reciprocal keeps the scalar engine free for
        # exp); copy the raw attention rows out of PSUM now so the av bank
        # frees for the next pair.  The normalize multiplies by an extra 64
        # to keep the fp8 context out of the denormal range; the out-proj
        # drain divides it back out.
        rec = sb.tile([P, T], F32, name=f"{tag}_rec{h}", tag="smrec", bufs=3)
        with nc.allow_low_precision(reason="fp32r view of an fp32 reciprocal"):
            nc.vector.reciprocal(_r(rec[64:65, :]), av[64:65, :])
        raw = sb.tile([64, T], F32, name=f"{tag}_raw{h}", tag="raw", bufs=3)
        nc.vector.tensor_copy(raw[:], av[0:64, :])

        def finish():
            pair, sub = h // 2, h % 2
            bc = ps.tile([P, T], F32, name=f"{tag}_bc{h}", tag="pj", bufs=2)
            _mm(nc, bc[0:64, :], ones[64:65, 0:64], rec[64:65, :])
            if sub == 0:
                nc.vector.scalar_tensor_tensor(
                    out8[0:64, ts(pair, T)], raw[:], 64.0, bc[0:64, :],
                    ALU.mult, ALU.mult,
                )
            else:
                # normalize at lanes 0..63, then DMA-migrate to lanes 64..127
                tmp = sb.tile([64, T], F8, name=f"{tag}_mig{h}", tag="omig",
                              bufs=1)
                nc.vector.scalar_tensor_tensor(
                    tmp[:], raw[:], 64.0, bc[0:64, :], ALU.mult, ALU.mult)
                nc.sync.dma_start(out8[64:128, ts(pair, T)], tmp[:])

        return finish

    for pair in range(H // 2):
        h0 = 2 * pair
        avs = [
            ps.tile([P, T], F32, name=f"{tag}_av{h0 + sub}", tag="av", bufs=2)
            for sub in range(2)
        ]
        for dj in range(SC // 2):
            sts = [
                ps.tile([P, 2 * T], F32, name=f"{tag}_st{h0 + sub}_{dj}",
                        tag="st", bufs=2)
                for sub in range(2)
            ]
            for half in range(2):
                j = 2 * dj + half
                for sub in range(2):
                    rb = 64 * sub
                    _mm(
                        nc, sts[sub][:, ts(half, T)],
                        kT[pair][rb:rb + 64, ts(j, P)],
                        qT[pair][rb:rb + 64, :],
                    )
                    if maskT is not None:
                        nc.vector.scalar_tensor_tensor(
                            sts[sub][:, ts(half, T)], sts[sub][:, ts(half, T)],
                            ISCALE, maskT[j][:], ALU.mult, ALU.add,
                        )
            for sub in range(2):
                pt = ptp.tile([P, 2 * T], F8, name=f"{tag}_pt{h0 + sub}_{dj}",
                              tag="pt", bufs=6)
                if maskT is not None:
                    nc.scalar.activation(pt[:], sts[sub][:], AF.Exp)
                else:
                    nc.scalar.activation(pt[:], sts[sub][:], AF.Exp,
                                         scale=ISCALE)
                h = h0 + sub
                vsl = V[dj].rearrange("p (k h x) -> p k h x", k=2, h=H)
                nc.tensor.matmul(
                    avs[sub][0:VW, :],
                    vsl[:, :, h, 0:VW],
                    pt[:].rearrange("p (k t) -> p k t", k=2),
                    start=(dj == 0), stop=(dj == SC // 2 - 1),
                    perf_mode=DR,
                )
            if dj == 1:
                for fin in pending:
                    fin()
                pending = []
            nxt = next(fill_iter, None)
            if nxt is not None:
                nxt()
        pending = [epilogue(h0, avs[0]), epilogue(h0 + 1, avs[1])]
    for fin in pending:
        fin()
    for nxt in fill_iter:
        nxt()


def build_program(flags, repeat=1):
    """Build and compile the Bass program.  flags keys: bias_qkv,
    bias_self_out, bias_kv, bias_q, bias_cross_out, bias_f1, bias_f2,
    ln1, ln2, ln3, mask_self, mask_cross.  repeat>1 unrolls the body
    multiple times (benchmarking only: amortizes dispatch overhead)."""
    nc = bacc.Bacc(
        "TRN2", target_bir_lowering=False, debug=False,
        num_devices=1, enable_asserts=False,
    )
    x_d = nc.dram_tensor("x", [S, D], F32, kind="ExternalInput").ap()
    y_d = nc.dram_tensor("y", [S, D], F32, kind="ExternalInput").ap()
    # weights arrive host-cast to fp8e4m3
    w_qkv = nc.dram_tensor("qkv_w", [D, 3 * D], F8, kind="ExternalInput").ap()
    w_so = nc.dram_tensor("self_out_w", [D, D], F8, kind="ExternalInput").ap()
    w_kv = nc.dram_tensor("kv_w", [D, 2 * D], F8, kind="ExternalInput").ap()
    w_q = nc.dram_tensor("q_w", [D, D], F8, kind="ExternalInput").ap()
    w_co = nc.dram_tensor("cross_out_w", [D, D], F8, kind="ExternalInput").ap()
    w_f1 = nc.dram_tensor("ffn_w1", [D, FH], BF16, kind="ExternalInput").ap()
    w_f2 = nc.dram_tensor("ffn_w2", [FH, D], BF16, kind="ExternalInput").ap()

    def opt_in(name, shape, flag):
        if flags[flag]:
            return nc.dram_tensor(name, shape, F32, kind="ExternalInput").ap()
        return None

    b_qkv_d = opt_in("qkv_b", [3 * D], "bias_qkv")
    b_so_d = opt_in("self_out_b", [D], "bias_self_out")
    b_kv_d = opt_in("kv_b", [2 * D], "bias_kv")
    b_q_d = opt_in("q_b", [D], "bias_q")
    b_co_d = opt_in("cross_out_b", [D], "bias_cross_out")
    b_f1_d = opt_in("ffn_b1", [FH], "bias_f1")
    b_f2_d = opt_in("ffn_b2", [D], "bias_f2")
    g1_d = opt_in("g1", [D], "ln1")
    b1_d = opt_in("b1", [D], "ln1")
    g2_d = opt_in("g2", [D], "ln2")
    b2_d = opt_in("b2", [D], "ln2")
    g3_d = opt_in("g3", [D], "ln3")
    b3_d = opt_in("b3", [D], "ln3")
    m_self_d = opt_in("mask_self", [T, S], "mask_self")
    m_cross_d = opt_in("mask_cross", [T, S], "mask_cross")

    out_d = nc.dram_tensor("out", [T, D], F32, kind="ExternalOutput").ap()

    with tile.TileContext(nc, pool_alloc_mode="queue") as tc_:
      for _rep in range(repeat):
       with ExitStack() as top:
        persist = top.enter_context(tc_.tile_pool(name="persist", bufs=1))

        ident = persist.tile([P, P], F32, name="ident")
        make_identity(nc, ident[:])
        ones_raw = persist.tile([P, P], F32, name="ones_raw")
        nc.vector.memset(ones_raw[:], 1.0)
        ones = persist.tile([P, P], F32, name="ones")
        nc.vector.tensor_copy(_r(ones[:]), ones_raw[:])
        tc_.ident = ident
        tc_.ones = ones
        tc_.ones_raw = ones_raw
        # dummy activation: forces the one ACT_TABLE_LOAD to run at t=0,
        # under the input DMA, instead of on the first-exp critical path
        actwarm = persist.tile([1, 1], F32, name="actwarm")
        nc.scalar.activation(actwarm[:], ones_raw[0:1, 0:1], AF.Exp)

        def load_vec_chunks(dram_ap, n, name):
            """[n] DRAM vector -> SBUF [P, n//P] (col c = chunk c)."""
            if dram_ap is None:
                return None
            t = persist.tile([P, n // P], F32, name=name)
            nc.sync.dma_start(t[:], dram_ap.rearrange("(c p) -> p c", p=P))
            return t

        b_qkv = load_vec_chunks(b_qkv_d, 3 * D, "b_qkv")
        b_so = load_vec_chunks(b_so_d, D, "b_so")
        b_kv = load_vec_chunks(b_kv_d, 2 * D, "b_kv")
        b_q = load_vec_chunks(b_q_d, D, "b_q")
        b_co = load_vec_chunks(b_co_d, D, "b_co")
        b_f1 = load_vec_chunks(b_f1_d, FH, "b_f1")
        b_f2 = load_vec_chunks(b_f2_d, D, "b_f2")
        g1 = load_vec_chunks(g1_d, D, "g1")
        b1 = load_vec_chunks(b1_d, D, "b1")
        g2 = load_vec_chunks(g2_d, D, "g2")
        b2 = load_vec_chunks(b2_d, D, "b2")
        g3 = load_vec_chunks(g3_d, D, "g3")
        b3 = load_vec_chunks(b3_d, D, "b3")

        y1_tiles = [persist.tile([P, T], F32, name=f"y1_{c}") for c in range(DC)]

        def gb_cols(g, b):
            if g is None:
                return None, None
            return (
                [g[:, c:c + 1] for c in range(DC)],
                [b[:, c:c + 1] for c in range(DC)],
            )

        def bias_bcast(sb_pool, ps_pool, src_ap, name):
            """Bias row (any AP of D elements in head order) broadcast across
            partitions -> [P, D]."""
            row = sb_pool.tile([P, D], F32, name=f"{name}_row")
            nc.sync.dma_start(_r(row[0:1, :]), _r(src_ap))
            bc_ps = ps_pool.tile([P, D], F32, name=f"{name}_ps", tag="pj")
            _mm(nc, bc_ps[:], ones[0:1, :], row[0:1, :])
            out = sb_pool.tile([P, D], F32, name=f"{name}_bc")
            nc.vector.tensor_copy(out[:], bc_ps[:])
            return out

        def proj_kT(ps, src8, w8, w_ncols, w_off, kT_o, bk_cols, tag, mc, ns,
                    drain):
            """One [P, T] tile of the feature-major K projection (fp8 DR)."""
            kp = ps.tile([P, T], F32, name=f"{tag}_kp{mc}_{ns}", tag="pj")
            s3 = _v3(src8, DC)
            w3 = _v3(w8, DC)
            for jp in range(DC // 2):
                nc.tensor.matmul(
                    kp[:],
                    w3[:, 2 * jp:2 * jp + 2,
                       w_off + mc * P:w_off + mc * P + P],
                    s3[:, 2 * jp:2 * jp + 2, ts(ns, T)],
                    start=(jp == 0), stop=(jp == DC // 2 - 1), perf_mode=DR,
                )
            dst = _r(kT_o[mc][:, ts(ns, T)])
            if drain == "act":
                if bk_cols is not None:
                    nc.scalar.activation(dst, kp[:], AF.Identity,
                                         bias=bk_cols[mc])
                else:
                    nc.scalar.copy(dst, kp[:])
            else:
                if bk_cols is not None:
                    nc.vector.tensor_scalar_add(dst, kp[:], bk_cols[mc])
                else:
                    nc.vector.tensor_copy(dst, kp[:])

        def v_ones_init(V_o):
            """Write the per-head softmax-denominator ones column of every V
            tile once (a cheap strided memset; the value drains never touch
            column HD)."""
            for v8 in V_o:
                nc.vector.memset(
                    v8[:].rearrange("p (k h x) -> p k h x", k=2, h=H)
                    [:, :, :, HD:HD + 1], 1.0)

        def proj_V(ps, src8, w8, v_off, V_o, vb_bc, tag, tc2, drain="dve"):
            """One key chunk of the fp8 V tiles: plane tc2%2 of pair tile
            tc2//2 (logical [P, 2, H, VWP]; ones column pre-written by
            v_ones_init)."""
            vp = ps.tile([P, D], F32, name=f"{tag}_vp{tc2}", tag="pj")
            s3 = _v3(src8, DC)
            w3 = _v3(w8, DC)
            for jp in range(DC // 2):
                nc.tensor.matmul(
                    vp[:],
                    s3[:, 2 * jp:2 * jp + 2, ts(tc2, P)],
                    w3[:, 2 * jp:2 * jp + 2, v_off:v_off + D],
                    start=(jp == 0), stop=(jp == DC // 2 - 1), perf_mode=DR,
                )
            vdst = V_o[tc2 // 2].rearrange(
                "p (k h x) -> p k h x", k=2, h=H)[:, tc2 % 2]
            if vb_bc is not None:
                nc.vector.tensor_add(
                    vdst[:, :, 0:HD], vp[:].rearrange("p (h x) -> p h x", h=H),
                    vb_bc[:].rearrange("p (h x) -> p h x", h=H),
                )
            elif drain == "act":
                nc.scalar.copy(
                    vdst[:, :, 0:HD], vp[:].rearrange("p (h x) -> p h x", h=H))
            else:
                nc.vector.tensor_copy(
                    vdst[:, :, 0:HD], vp[:].rearrange("p (h x) -> p h x", h=H))

        def proj_qT(ps, src8, src_ncols, w8, q_off, qT_o, bq_cols, tag,
                    mcs=None):
            s3 = _v3(src8, src_ncols)
            w3 = _v3(w8, DC)
            for mc in (range(DC) if mcs is None else mcs):
                qp = ps.tile([P, T], F32, name=f"{tag}_qp{mc}", tag="pj")
                for jp in range(DC // 2):
                    nc.tensor.matmul(
                        qp[:],
                        w3[:, 2 * jp:2 * jp + 2,
                           q_off + mc * P:q_off + mc * P + P],
                        s3[:, 2 * jp:2 * jp + 2, 0:T],
                        start=(jp == 0), stop=(jp == DC // 2 - 1),
                        perf_mode=DR,
                    )
                if bq_cols is not None:
                    nc.scalar.activation(_r(qT_o[mc][:]), qp[:], AF.Identity,
                                         bias=bq_cols[mc])
                else:
                    nc.scalar.copy(_r(qT_o[mc][:]), qp[:])

        def out_proj_residual(ps_blk, w8, attn8, bias, resid, r_out):
            a3 = _v3(attn8, DC)
            w3 = _v3(w8, DC)
            for mc in range(DC):
                op = ps_blk.tile([P, T], F32, name=f"op{mc}", tag="pj")
                for jp in range(DC // 2):
                    nc.tensor.matmul(
                        op[:],
                        w3[:, 2 * jp:2 * jp + 2, ts(mc, P)],
                        a3[:, 2 * jp:2 * jp + 2, :],
                        start=(jp == 0), stop=(jp == DC // 2 - 1),
                        perf_mode=DR,
                    )
                # attention context was written scaled by 64 (fp8 range);
                # undo it here while adding the residual.
                nc.vector.scalar_tensor_tensor(
                    _r(r_out[mc][:]), op[:], 1.0 / 64.0,
                    resid[mc][:], ALU.mult, ALU.add,
                )
                if bias is not None:
                    nc.vector.tensor_scalar_add(
                        _r(r_out[mc][:]), r_out[mc][:], bias[:, mc:mc + 1])

        # ==================== SELF-ATTENTION BLOCK ====================
        sa_kv_blk = ExitStack()
        sa_sb = top.enter_context(tc_.tile_pool(name="sa_sb", bufs=1))
        sa_kv = sa_kv_blk.enter_context(tc_.tile_pool(name="sa_kv", bufs=1))
        kT_s = [sa_kv.tile([P, S], F32, name=f"kTs{c}") for c in range(DC)]
        V_s = [sa_kv.tile([P, 2 * H * VWP], F8, name=f"Vs{j}")
               for j in range(SC // 2)]
        qT_s = [sa_kv.tile([P, T], F32, name=f"qTs{c}") for c in range(DC)]
        attn_p_s = sa_sb.tile([P, DC * T], F8, name="attnPs")
        y_res = [sa_sb.tile([P, T], F32, name=f"yres{c}") for c in range(DC)]
        maskT_s = None
        if m_self_d is not None:
            maskT_s = _transpose_mask(nc, tc_, sa_kv, m_self_d, "ms")

        yT8 = sa_kv.tile([P, DC * S], F8, name="yT8")
        # qkv_w/qkv_b arrive host-permuted to [all-q | all-k | all-v],
        # head-major inside each section -> contiguous slices here.  The K
        # section loads first: it heads the first-score critical chain.
        wq8 = sa_kv.tile([P, DC * 3 * D], F8, name="wqkv8")
        _wq3d = wq8[:].rearrange("p (c m) -> p c m", c=DC)
        _wqd3 = w_qkv.rearrange("(c p) m -> p c m", p=P)
        nc.sync.dma_start(_wq3d[:, :, D:2 * D], _wqd3[:, :, D:2 * D])
        vb_bc = None
        bk_cols = bq_cols = None
        if b_qkv is not None:
            with tc_.tile_pool(name="vb_ps", bufs=1, space="PSUM") as vps:
                vb_bc = bias_bcast(
                    sa_sb, vps, b_qkv_d[2 * D:3 * D].rearrange("(a n) -> a n", a=1),
                    "vb_s")
            bk_cols = [b_qkv[:, DC + mc:DC + mc + 1] for mc in range(DC)]
            bq_cols = [b_qkv[:, mc:mc + 1] for mc in range(DC)]
        no_mask = m_self_d is None and m_cross_d is None
        v_ones_init(V_s)
        if no_mask:
            # Minimal pre-attention prefix: only what pair 0's first two
            # double-chunks need (y tokens 0:512 transposed, kT(pair0, first
            # 512 keys), all queries, V pairs 0-1).  Everything else streams
            # in as deadline-ordered filler slots inside the attention phase,
            # so the first exp fires after ~1.75MB of DMA instead of ~5MB.
            with tc_.tile_pool(name="sa_tm", bufs=4) as tm_pool, \
                 tc_.tile_pool(name="sa_tp", bufs=4, space="PSUM") as tp_ps:
                o3 = _v3(yT8, DC)
                for sc in range(TC):
                    tm = tm_pool.tile([P, D], F32, name=f"y_tm{sc}",
                                      tag="y_tm")
                    nc.sync.dma_start(tm[:], y_d[ts(sc, P), :])
                    for dc in range(DC):
                        tp = tp_ps.tile([P, P], F32, name=f"y_tp{sc}_{dc}",
                                        tag="tp")
                        nc.tensor.transpose(tp[:], tm[:, ts(dc, P)], ident[:])
                        nc.vector.tensor_copy(o3[:, dc, ts(sc, P)], tp[:])
                        if sc < TC:
                            nc.vector.tensor_copy(
                                y_res[dc][:, ts(sc, P)], tp[:])
            nc.sync.dma_start(_wq3d[:, :, 0:D], _wqd3[:, :, 0:D])
            nc.sync.dma_start(_wq3d[:, :, 2 * D:3 * D], _wqd3[:, :, 2 * D:3 * D])
            with tc_.tile_pool(name="sa_prj_ps", bufs=4, space="PSUM") as ps:
                proj_kT(ps, yT8, wq8, 3 * D, D, kT_s, bk_cols, "sa", 0, 0,
                        drain="act")
                proj_qT(ps, yT8, DC, wq8, 0, qT_s, bq_cols, "sa")
                for tc2 in range(4):
                    proj_V(ps, yT8, wq8, 2 * D, V_s, vb_bc, "sa", tc2)
        else:
            with tc_.tile_pool(name="sa_tm", bufs=4) as tm_pool, \
                 tc_.tile_pool(name="sa_tp", bufs=4, space="PSUM") as tp_ps:
                _transpose_in(nc, tc_, tm_pool, tp_ps, yT8, y_d, S, "y",
                              res_tiles=y_res)
            with tc_.tile_pool(name="sa_prj_ps", bufs=4, space="PSUM") as ps:
                for mc in range(DC):
                    for ns in range(SC // DC):
                        proj_kT(ps, yT8, wq8, 3 * D, D, kT_s, bk_cols, "sa",
                                mc, ns, drain="act")
                proj_qT(ps, yT8, DC, wq8, 0, qT_s, bq_cols, "sa")
                for tc2 in range(SC):
                    proj_V(ps, yT8, wq8, 2 * D, V_s, vb_bc, "sa", tc2)

        # Cross-attention prep (x transposes + cross K projection) is
        # independent of self-attention; in the no-mask variant it is
        # interleaved into the self-attention phase as PE filler work.
        xw_blk = ExitStack()
        ca_kT_blk = ExitStack()
        _cross_prep = {}

        def setup_cross_prep():
            ca_kT_pool = ca_kT_blk.enter_context(
                tc_.tile_pool(name="ca_kT", bufs=1, side="right"))
            kT_c = [ca_kT_pool.tile([P, S], F32, name=f"kTc{c}")
                    for c in range(DC)]
            xw_sb = xw_blk.enter_context(
                tc_.tile_pool(name="xw_sb", bufs=1, side="right"))
            xT8 = xw_sb.tile([P, DC * S], F8, name="xT8")
            wkv8 = _load_w8(nc, xw_sb, w_kv, D, 2 * D, "wkv8")
            _cross_prep["kT_c"] = kT_c
            _cross_prep["xT8"] = xT8
            _cross_prep["wkv8"] = wkv8
            return kT_c, xT8, wkv8

        bk_cols_c = None
        if b_kv is not None:
            bk_cols_c = [b_kv[:, mc:mc + 1] for mc in range(DC)]

        def make_filler(ptp, aps):
            """32 filler slots (one consumed at the end of each attention
            double-chunk iteration).  Slot s is emitted before iteration s+1,
            so every unit sits ahead of its first consumer in the PE queue:
            y-transpose sc before kT(*, sc//4) / V(sc//2) in the same or a
            later slot; V pair j before pair-0 AV of chunk j (iteration j);
            kT(p, ns) before pair p's scores on keys ns*512.. (iteration
            8p+2ns); x-transposes/cross-K only feed the (later) cross
            attention."""
            kT_c, xT8 = _cross_prep["kT_c"], _cross_prep["xT8"]
            wkv8 = _cross_prep["wkv8"]
            xtm_pool = ptp  # token-major staging tiles share the pt pool
            x3 = _v3(xT8, DC)
            y3 = _v3(yT8, DC)

            def ytr(sc):
                def emit():
                    tm = xtm_pool.tile([P, D], F32, name=f"y_tm{sc}",
                                       tag="xtm", bufs=4)
                    nc.sync.dma_start(tm[:], y_d[ts(sc, P), :])
                    for dc in range(DC):
                        tp = aps.tile([P, P], F32, name=f"y_tp{sc}_{dc}",
                                      tag="pj", bufs=2)
                        nc.tensor.transpose(tp[:], tm[:, ts(dc, P)],
                                            tc_.ident[:])
                        nc.vector.tensor_copy(y3[:, dc, ts(sc, P)], tp[:])
                return emit

            def vs(jp):
                def emit():
                    proj_V(aps, yT8, wq8, 2 * D, V_s, vb_bc, "sa", 2 * jp)
                    proj_V(aps, yT8, wq8, 2 * D, V_s, vb_bc, "sa", 2 * jp + 1)
                return emit

            def kts(mc, ns):
                def emit():
                    proj_kT(aps, yT8, wq8, 3 * D, D, kT_s, bk_cols, "sa",
                            mc, ns, drain="dve")
                return emit

            def qts(mc):
                def emit():
                    proj_qT(aps, yT8, DC, wq8, 0, qT_s, bq_cols, "sa",
                            mcs=[mc])
                return emit

            def xtr(sc):
                def emit():
                    tm = xtm_pool.tile([P, D], F32, name=f"x_tm{sc}",
                                       tag="xtm", bufs=4)
                    nc.sync.dma_start(tm[:], x_d[ts(sc, P), :])
                    for dc in range(DC):
                        tp = aps.tile([P, P], F32, name=f"x_tp{sc}_{dc}",
                                      tag="pj", bufs=2)
                        nc.tensor.transpose(tp[:], tm[:, ts(dc, P)],
                                            tc_.ident[:])
                        nc.vector.tensor_copy(x3[:, dc, ts(sc, P)], tp[:])
                return emit

            def kc(mc, ns):
                def emit():
                    proj_kT(aps, xT8, wkv8, 2 * D, 0, kT_c, bk_cols_c, "ca",
                            mc, ns, drain="dve")
                return emit

            # within a slot: score-critical kT first (its DVE drain must
            # not queue behind the V drains), transposes next, V last; the
            # y-transpose stream is front-loaded so each kT lands 1-2 slots
            # before its consumer instead of exactly at the deadline
            slots = [
                [ytr(4), ytr(5), ytr(6), ytr(7), vs(2)],
                [kts(0, 1), ytr(8), ytr(9), ytr(10), ytr(11), vs(3)],
                [kts(0, 2), ytr(12), ytr(13), ytr(14), ytr(15), vs(4)],
                [kts(0, 3), vs(5)],
                [kts(1, 0), vs(6)],
                [kts(1, 1), vs(7)],
                [kts(1, 2)],
                [kts(1, 3)],
                [xtr(0), xtr(1)],
                [xtr(2), xtr(3)],
                [kts(2, 0), xtr(4), xtr(5)],
                [kts(2, 1), xtr(6), xtr(7)],
                [kts(2, 2), xtr(8), xtr(9)],
                [kts(2, 3), xtr(10), xtr(11)],
                [kts(3, 0), xtr(12), xtr(13)],
                [kts(3, 1), xtr(14), xtr(15)],
                [kts(3, 2), kc(0, 0)],
                [kts(3, 3), kc(1, 0)],
                [kc(2, 0)], [kc(3, 0)],
                [kc(0, 1)], [kc(1, 1)], [kc(2, 1)], [kc(3, 1)],
                [kc(0, 2)], [kc(1, 2)], [kc(2, 2)], [kc(3, 2)],
                [kc(0, 3)], [kc(1, 3)], [kc(2, 3)], [kc(3, 3)],
            ]

            def slotfn(lst):
                def emit():
                    for u in lst:
                        u()
                return emit

            return [slotfn(lst) for lst in slots]

        # prefetch the small out-proj / cross-q weights so the inter-phase
        # stretch never waits on their DMA
        wso8 = _load_w8(nc, sa_sb, w_so, D, D, "wso8")
        wqc8 = _load_w8(nc, sa_sb, w_q, D, D, "wqc8")
        wco8 = _load_w8(nc, sa_sb, w_co, D, D, "wco8")

        if m_cross_d is None and m_self_d is None:
            kT_c, xT8, wkv8 = setup_cross_prep()
            with tc_.tile_pool(name="att_s_sb", bufs=1) as asb, \
                 tc_.tile_pool(name="att_s_pt", bufs=1) as aptp, \
                 tc_.tile_pool(name="att_s_ps", bufs=2, space="PSUM") as aps:
                _attention(nc, tc_, asb, aptp, aps, kT_s, V_s, qT_s, maskT_s,
                           attn_p_s, "sa", filler=make_filler(aptp, aps))
        else:
            # mask variant: no interleaving (SBUF budget goes to mask tiles)
            with tc_.tile_pool(name="att_s_sb", bufs=1) as asb, \
                 tc_.tile_pool(name="att_s_pt", bufs=1) as aptp, \
                 tc_.tile_pool(name="att_s_ps", bufs=2, space="PSUM") as aps:
                _attention(nc, tc_, asb, aptp, aps, kT_s, V_s, qT_s, maskT_s,
                           attn_p_s, "sa")
            sa_kv_blk.close()
            kT_c, xT8, wkv8 = setup_cross_prep()
            with tc_.tile_pool(name="xp_tm", bufs=4) as tm_pool, \
                 tc_.tile_pool(name="xp_ps", bufs=4, space="PSUM") as tp_ps:
                _transpose_in(nc, tc_, tm_pool, tp_ps, xT8, x_d, S, "x")
                for mc in range(DC):
                    for ns in range(SC // DC):
                        proj_kT(tp_ps, xT8, wkv8, 2 * D, 0, kT_c, bk_cols_c,
                                "ca", mc, ns, drain="dve")

        sa_kv_blk.close()
        y2_pool = top.enter_context(tc_.tile_pool(name="y2_pool", bufs=1))
        y2_tiles = [y2_pool.tile([P, T], F32, name=f"y2_{c}") for c in range(DC)]

        # Cross V projection + pools hoisted before LN1: its matmuls fill
        # the PE gap while LN1's scalar/vector chain runs (separate PSUM
        # pool avoids the bank-reuse WAR with LN1's stats tiles).
        blk = ExitStack()
        ca_sb = blk.enter_context(tc_.tile_pool(name="ca_sb", bufs=1))
        ca_v_sb = blk.enter_context(tc_.tile_pool(name="ca_v_sb", bufs=1))
        V_c = [ca_v_sb.tile([P, 2 * H * VWP], F8, name=f"Vc{j}")
               for j in range(SC // 2)]
        qT_c = [ca_sb.tile([P, T], F32, name=f"qTc{c}") for c in range(DC)]
        attn_p_c = ca_sb.tile([P, DC * T], F8, name="attnPc")
        maskT_c = None
        vb_bc_c = None
        bq_cols_c = None
        if b_kv is not None:
            with tc_.tile_pool(name="vbc_ps", bufs=1, space="PSUM") as vps:
                vb_bc_c = bias_bcast(
                    ca_sb, vps, b_kv_d[D:2 * D].rearrange("(a n) -> a n", a=1),
                    "vb_c")
        if b_q is not None:
            bq_cols_c = [b_q[:, mc:mc + 1] for mc in range(DC)]
        ca_ps_blk = ExitStack()
        ca_prj_ps = ca_ps_blk.enter_context(
            tc_.tile_pool(name="ca_prj_ps", bufs=4, space="PSUM"))
        v_ones_init(V_c)
        for tc2 in range(SC):
            proj_V(ca_prj_ps, xT8, wkv8, D, V_c, vb_bc_c, "ca", tc2,
                   drain="act")
        xw_blk.close()

        with tc_.tile_pool(name="ph4_sb", bufs=1) as p4sb, \
             tc_.tile_pool(name="ph4_ps", bufs=2, space="PSUM") as p4ps:
            r1 = [p4sb.tile([P, T], F32, name=f"r1_{c}") for c in range(DC)]
            out_proj_residual(p4ps, wso8, attn_p_s, b_so, y_res, r1)
            g1c, b1c = gb_cols(g1, b1)
            _feat_layernorm(nc, tc_, p4sb, p4ps, r1, y1_tiles, g1c, b1c, "ln1")
            y18 = ca_sb.tile([P, DC * T], F8, name="y18")
            for c in range(DC):
                eng = nc.scalar.copy if c % 2 == 0 else nc.vector.tensor_copy
                eng(_v3(y18, DC)[:, c, :], y1_tiles[c][:])
            proj_qT(ca_prj_ps, y18, DC, wqc8, 0, qT_c, bq_cols_c, "ca")
        ca_ps_blk.close()

        # FFN weights prefetch: the 6MB bf16 load overlaps the (scalar-
        # engine-bound) cross-attention phase instead of stalling the tail.
        # Allocated from the long-lived y2_pool to keep pool stack order.
        wf1b = y2_pool.tile([P, DC * FH], BF16, name="wf1b")
        nc.sync.dma_start(
            _v3(wf1b, DC), w_f1.rearrange("(c p) m -> p c m", p=P))
        wf2b = y2_pool.tile([P, FC * D], BF16, name="wf2b")
        nc.sync.dma_start(
            _v3(wf2b, FC), w_f2.rearrange("(c p) m -> p c m", p=P))

        # ==================== CROSS-ATTENTION BLOCK ====================
        if True:
            if m_cross_d is not None:
                mc_pool = blk.enter_context(
                    tc_.tile_pool(name="mc_pool", bufs=1))
                maskT_c = _transpose_mask(nc, tc_, mc_pool, m_cross_d, "mc")

            with tc_.tile_pool(name="att_c_sb", bufs=1) as asb, \
                 tc_.tile_pool(name="att_c_pt", bufs=1) as aptp, \
                 tc_.tile_pool(name="att_c_ps", bufs=2, space="PSUM") as aps:
                _attention(nc, tc_, asb, aptp, aps, kT_c, V_c, qT_c, maskT_c,
                           attn_p_c, "ca")
            ca_kT_blk.close()

            with tc_.tile_pool(name="ph5d_sb", bufs=1) as p5sb, \
                 tc_.tile_pool(name="ph5d_ps", bufs=2, space="PSUM") as p5ps:
                r2 = [p5sb.tile([P, T], F32, name=f"r2_{c}") for c in range(DC)]
                out_proj_residual(p5ps, wco8, attn_p_c, b_co, y1_tiles, r2)
                g2c, b2c = gb_cols(g2, b2)
                _feat_layernorm(nc, tc_, p5sb, p5ps, r2, y2_tiles, g2c, b2c, "ln2")
        blk.close()

        # ==================== FFN + LN3 + STORE ====================
        # FFN runs in fp32r: same PE rate as bf16 at these tile widths, but
        # the LN2 output feeds the matmuls directly (no cast on the critical
        # chain) and the rounding error is ~fp22 instead of bf16.
        with tc_.tile_pool(name="ph6_sb", bufs=1) as p6sb, \
             tc_.tile_pool(name="ph6_ps", bufs=2, space="PSUM") as p6ps:
            y2b = p6sb.tile([P, DC * T], BF16, name="y2b")
            for c in range(DC):
                eng = nc.scalar.copy if c % 2 == 0 else nc.vector.tensor_copy
                eng(_v3(y2b, DC)[:, c, :], y2_tiles[c][:])
            y2b3 = _v3(y2b, DC)
            wf13 = _v3(wf1b, DC)
            wf23 = _v3(wf2b, FC)
            hb = p6sb.tile([P, FC * T], BF16, name="hb")
            hb3 = _v3(hb, FC)
            for fc in range(FC):
                hp = p6ps.tile([P, T], F32, name=f"hps{fc}", tag="pj")
                for kc in range(DC):
                    nc.tensor.matmul(
                        hp[:],
                        wf13[:, kc, ts(fc, P)],
                        y2b3[:, kc, :],
                        start=(kc == 0), stop=(kc == DC - 1),
                    )
                if b_f1 is not None:
                    nc.scalar.activation(
                        hb3[:, fc, :], hp[:], AF.Relu, bias=b_f1[:, fc:fc + 1]
                    )
                else:
                    nc.scalar.activation(hb3[:, fc, :], hp[:], AF.Relu)
            r3 = [p6sb.tile([P, T], F32, name=f"r3_{c}") for c in range(DC)]
            for mc in range(DC):
                op = p6ps.tile([P, T], F32, name=f"fop{mc}", tag="pj")
                for fc in range(FC):
                    nc.tensor.matmul(
                        op[:],
                        wf23[:, fc, ts(mc, P)],
                        hb3[:, fc, :],
                        start=(fc == 0), stop=(fc == FC - 1),
                    )
                if b_f2 is not None:
                    nc.vector.scalar_tensor_tensor(
                        r3[mc][:], op[:], b_f2[:, mc:mc + 1], y2_tiles[mc][:],
                        ALU.add, ALU.add,
                    )
                else:
                    nc.vector.tensor_add(r3[mc][:], op[:], y2_tiles[mc][:])

            # LN3 token-major: transpose r3, bn_stats, normalize, store
            # (pipelines per token chunk; beats a feature-major pass here).
            g3bc = b3bc = None
            if g3 is not None:
                g3row = p6sb.tile([P, D], F32, name="g3row")
                nc.sync.dma_start(_r(g3row[0:1, :]), _r(g3_d.rearrange("(a n) -> a n", a=1)))
                b3row = p6sb.tile([P, D], F32, name="b3row")
                nc.sync.dma_start(_r(b3row[0:1, :]), _r(b3_d.rearrange("(a n) -> a n", a=1)))
                g3ps = p6ps.tile([P, D], F32, name="g3ps", tag="pj")
                _mm(nc, g3ps[:], ones[0:1, :], g3row[0:1, :])
                g3bc = p6sb.tile([P, D], F32, name="g3bc")
                nc.vector.tensor_copy(g3bc[:], g3ps[:])
                b3ps = p6ps.tile([P, D], F32, name="b3ps", tag="pj")
                _mm(nc, b3ps[:], ones[0:1, :], b3row[0:1, :])
                b3bc = p6sb.tile([P, D], F32, name="b3bc")
                nc.vector.tensor_copy(b3bc[:], b3ps[:])

            for tc2 in range(TC):
                rtm = p6sb.tile([P, D], F32, name=f"rtm{tc2}", tag="rtm", bufs=3)
                for dc in range(DC):
                    tp = p6ps.tile([P, P], F32, name=f"otp{tc2}_{dc}", tag="otp")
                    nc.tensor.transpose(tp[:], r3[dc][:, ts(tc2, P)], ident[:])
                    nc.vector.tensor_copy(rtm[:, ts(dc, P)], tp[:])
                st6 = p6sb.tile([P, 6], F32, name=f"st6_{tc2}", tag="st6", bufs=3)
                nc.vector.bn_stats(st6[:], rtm[:])
                mv = p6sb.tile([P, 2], F32, name=f"mv{tc2}", tag="mv", bufs=3)
                nc.vector.bn_aggr(mv[:], st6[:])
                # rstd = 1/sqrt(var*D/(D-1)) (eps fold: see _feat_layernorm)
                lnv = p6sb.tile([P, 1], F32, name=f"olnv{tc2}", tag="osm", bufs=3)
                nc.scalar.activation(
                    lnv[:], mv[:, 1:2], AF.Ln, scale=float(D) / (D - 1)
                )
                rstd = p6sb.tile([P, 1], F32, name=f"orstd{tc2}", tag="osm4", bufs=3)
                nc.scalar.activation(rstd[:], lnv[:], AF.Exp, scale=-0.5)
                otm = p6sb.tile([P, D], F32, name=f"otm{tc2}", tag="otm", bufs=3)
                nc.vector.tensor_scalar(
                    otm[:], rtm[:], mv[:, 0:1], rstd[:], ALU.subtract, ALU.mult
                )
                if g3bc is not None:
                    nc.vector.tensor_mul(otm[:], otm[:], g3bc[:])
                    nc.vector.tensor_add(otm[:], otm[:], b3bc[:])
                nc.sync.dma_start(out_d[ts(tc2, P), :], otm[:])

    nc.compile()
    return nc


_PROGRAM_CACHE = {}


def _get_program(flags):
    key = tuple(sorted(flags.items()))
    if key not in _PROGRAM_CACHE:
        _PROGRAM_CACHE[key] = build_program(flags)
    return _PROGRAM_CACHE[key]


def make_flags(inputs):
    def nz(name):
        return bool(np.any(np.asarray(inputs[name])))

    return {
        "bias_qkv": nz("qkv_b"),
        "bias_self_out": nz("self_out_b"),
        "bias_kv": nz("kv_b"),
        "bias_q": nz("q_b"),
        "bias_cross_out": nz("cross_out_b"),
        "bias_f1": nz("ffn_b1"),
        "bias_f2": nz("ffn_b2"),
        "ln1": bool(np.any(np.asarray(inputs["g1"]) != 1.0) or nz("b1")),
        "ln2": bool(np.any(np.asarray(inputs["g2"]) != 1.0) or nz("b2")),
        "ln3": bool(np.any(np.asarray(inputs["g3"]) != 1.0) or nz("b3")),
        "mask_self": nz("self_attention_mask"),
        "mask_cross": nz("cross_attention_mask"),
    }


def make_in_maps(inputs, flags):
    """Per-core input dicts.  Core c handles batch c//4, query block c%4;
    x/y are rolled so the query block is first."""
    inputs = {k: np.asarray(v) for k, v in inputs.items()}
    # The reference splits qkv per head (reshape(B,S,H,3*HD) then split on the
    # last axis), i.e. qkv_w columns are [q_h|k_h|v_h] blocks of HD per head.
    # Permute on the host to [all-q | all-k | all-v] (head-major inside each
    # section) so the device uses contiguous slices.
    qkv_w = inputs["qkv_w"].reshape(D, H, 3, HD).transpose(0, 2, 1, 3).reshape(D, 3 * D)
    qkv_b = inputs["qkv_b"].reshape(H, 3, HD).transpose(1, 0, 2).reshape(3 * D)
    kv_w = inputs["kv_w"].reshape(D, H, 2, HD).transpose(0, 2, 1, 3).reshape(D, 2 * D)
    kv_b = inputs["kv_b"].reshape(H, 2, HD).transpose(1, 0, 2).reshape(2 * D)
    inputs = {**inputs, "qkv_w": qkv_w, "qkv_b": qkv_b, "kv_w": kv_w, "kv_b": kv_b}
    # attention matmuls run in fp8e4m3 DoubleRow, the FFN in bf16: cast the
    # weights on the host (this also cuts weight HBM traffic 4x / 2x).
    f8 = mybir.dt.np(F8)
    bf16 = mybir.dt.np(BF16)
    for wname in ("qkv_w", "kv_w", "q_w", "self_out_w", "cross_out_w"):
        inputs[wname] = np.asarray(inputs[wname], np.float32).astype(f8)
    for wname in ("ffn_w1", "ffn_w2"):
        inputs[wname] = np.asarray(inputs[wname], np.float32).astype(bf16)
    in_maps = []
    for c in range(N_CORES):
        b, qb = c // 4, c % 4
        qoff = qb * T
        m = {
            "x": np.roll(inputs["x"][b], -qoff, axis=0),
            "y": np.roll(inputs["y"][b], -qoff, axis=0),
            "qkv_w": inputs["qkv_w"],
            "self_out_w": inputs["self_out_w"],
            "kv_w": inputs["kv_w"],
            "q_w": inputs["q_w"],
            "cross_out_w": inputs["cross_out_w"],
            "ffn_w1": inputs["ffn_w1"],
            "ffn_w2": inputs["ffn_w2"],
        }
        for flag, names in (
            ("bias_qkv", ["qkv_b"]), ("bias_self_out", ["self_out_b"]),
            ("bias_kv", ["kv_b"]), ("bias_q", ["q_b"]),
            ("bias_cross_out", ["cross_out_b"]),
            ("bias_f1", ["ffn_b1"]), ("bias_f2", ["ffn_b2"]),
            ("ln1", ["g1", "b1"]), ("ln2", ["g2", "b2"]), ("ln3", ["g3", "b3"]),
        ):
            if flags[flag]:
                for n in names:
                    m[n] = inputs[n]
        if flags["mask_self"]:
            msk = np.broadcast_to(inputs["self_attention_mask"], (1, 1, S, S))[0, 0]
            m["mask_self"] = np.roll(msk[qoff:qoff + T, :], -qoff, axis=1)
        if flags["mask_cross"]:
            msk = np.broadcast_to(inputs["cross_attention_mask"], (1, 1, S, S))[0, 0]
            m["mask_cross"] = np.roll(msk[qoff:qoff + T, :], -qoff, axis=1)
        m = {
            k: (np.ascontiguousarray(v) if v.dtype in (f8, bf16)
                else np.ascontiguousarray(v, dtype=np.float32))
            for k, v in m.items()
        }
        in_maps.append(m)
    return in_maps


def assemble_output(results):
    out = np.empty((B, S, D), np.float32)
    for c in range(N_CORES):
        b, qb = c // 4, c % 4
        out[b, qb * T:(qb + 1) * T, :] = results[c]["out"]
    return out


def kernel(**inputs) -> np.ndarray:
    flags = make_flags(inputs)
    nc = _get_program(flags)
    in_maps = make_in_maps(inputs, flags)
    last_err = None
    for attempt in range(3):
        try:
            res = bass_utils.run_bass_kernel_spmd(
                nc, in_maps, core_ids=list(range(N_CORES))
            )
            return assemble_output(res.results)
        except Exception as e:  # transient NRT device errors observed on axon
            last_err = e
            if "UNRECOVERABLE" not in str(e) and "UNAVAILABLE" not in str(e):
                raise
    raise last_err

